# revision 30
# baseline (speedup 1.0000x reference)
"""Trainium2 Bass kernel for CrossInferBlock (spatial+temporal cross attention
+ out-projection + residual + BatchNorm over (B,T,N)).

Sharding: data-parallel over B across 8 NeuronCores (one batch element per
core). BN batch statistics are all-reduced across cores (8KB collective).

All matmuls run in bf16 (fp32 PSUM accumulate); residual/stats/BN in fp32.

Token orders: x and the projections (thT/phT/g) use ACTOR-MAJOR order
(tok = j*T + t) so every matmul operand is a legal single-stride access
pattern: temporal groups (8 actors x 16 timesteps) are contiguous 128-token
slices, spatial slices (one timestep, all 128 actors) are stride-T slices.
stT and the output use TIME-MAJOR order (tok = t*N + j): the temporal apply
pays a strided scatter once per group (hidden under the long projection
phase, split across ACT and DVE), which makes the phase-2 spatial adds
contiguous AND lets the out-projection start per 512-token time-chunk as
soon as its 4 spatial slices have landed.

g is projected ONCE (actor-group tiles, stationary operand read straight
from x); the 16 per-timestep tiles needed by the spatial applies are derived
with partition-gather SBUF->SBUF DMAs instead of a second projection pass.

The BN stats collective is split 7/1 so the first AllReduce's rendezvous
overlaps the out-projection tail and the bulk BN apply overlaps the second;
BN apply+store is split across the ACT and DVE engines with bf16 stores on
two HWDGE rings (the host upcasts to fp32).

All DRAM tensors are pre-tiled host-side to [128, X] exactly matching their
SBUF destination so every load is a full-row contiguous DMA.
"""

import sys

if "/opt/trn_rl_repo" not in sys.path:
    sys.path.insert(0, "/opt/trn_rl_repo")

import numpy as np
import ml_dtypes

import concourse.bass as bass
import concourse.bacc as bacc
import concourse.tile as tile
import concourse.mybir as mybir
from concourse.bass_utils import run_bass_kernel_spmd
from contextlib import ExitStack

F32 = mybir.dt.float32
BF16 = mybir.dt.bfloat16
F8 = mybir.dt.float8e4
DR = mybir.MatmulPerfMode.DoubleRow
AX = mybir.AxisListType
OP = mybir.AluOpType
ACT_FN = mybir.ActivationFunctionType

N_CORES = 8
B, T, N, C = 8, 16, 128, 1024
L = C // 2            # 512
TOK = T * N           # 2048 tokens per batch element
NTOK_GLOBAL = B * T * N
JG = 8                # actors per temporal group
NGRP = N // JG        # 16 groups
BN_EPS = 1e-5

SP_SCALE = 1.0 / (N * (T + N))   # spatial: /N then /(T+N)
TP_SCALE = 1.0 / (T * (T + N))   # temporal: /T then /(T+N)

NCC = C // 128     # 8 c-chunks
NLC = L // 128     # 4 l-chunks
NTC = TOK // 512   # 4 token chunks
CT_SPLIT = 7       # channel tiles covered by the first stats collective

_compiled = None
_last_results = None

USE_COLLECTIVE = True
USE_GATHER = True        # g_sp via SBUF->SBUF partition-gather DMA
USE_FP8 = True           # fp8e4 DoubleRow for theta/g projections
PH_BF16 = True           # phi stays bf16 (recovers quantization margin)
USE_FP8_OUT = True       # fp8e4 DoubleRow out-projection (stT + Ww in fp8)
W_SCALE = 16.0           # host premultiplies Wt/Wg (avoids fp8 subnormals)
ST_SCALE = 16.0 if USE_FP8_OUT else 1.0   # stT stored pre-scaled in fp8
# th x16, ph x16 (or x1 if PH_BF16), g x16 -> tw/sw and tp/sp carry the
# product; stT absorbs ST_SCALE
_PROJ = W_SCALE * W_SCALE * (1.0 if PH_BF16 else W_SCALE)
ATT_DESCALE = ST_SCALE / _PROJ



def ts(i, size):
    return bass.ts(i, size)


def _build():
    nc = bacc.Bacc("TRN2", target_bir_lowering=False, debug=False,
                   num_devices=N_CORES)

    # ---- DRAM I/O (pre-tiled [128, X]) ----
    # xbf rows: partition p; cols (tck, c, k): actor-major tokens,
    # x[c*128+p, tck*512+k] with tok = j*T + t
    XDT = F8 if USE_FP8 else BF16
    xbf_d = nc.dram_tensor("xbf", [128, NTC * NCC * 512], XDT,
                           kind="ExternalInput")
    xres_d = nc.dram_tensor("xres", [128, NTC * NCC * 512], BF16,
                            kind="ExternalInput")
    wt_d = nc.dram_tensor("wt", [128, NCC * L], XDT, kind="ExternalInput")
    wp_d = nc.dram_tensor("wp", [128, NCC * L],
                          BF16 if PH_BF16 else XDT, kind="ExternalInput")
    wg_d = nc.dram_tensor("wg", [128, NCC * L], XDT, kind="ExternalInput")
    ww_d = nc.dram_tensor("ww", [128, NLC * C],
                          F8 if USE_FP8_OUT else BF16, kind="ExternalInput")
    mask_d = nc.dram_tensor("mask", [128, 128], BF16, kind="ExternalInput")
    gb_d = nc.dram_tensor("gb", [128, 16], F32, kind="ExternalInput")
    # outy rows: partition p; cols (ct, tok): TIME-major tokens
    outy_d = nc.dram_tensor("outy", [128, NCC * TOK], BF16,
                            kind="ExternalOutput")

    with tile.TileContext(nc) as tc:
        with ExitStack() as outer:
            # ---------------- persistent pools ----------------
            cpool = outer.enter_context(tc.tile_pool(name="consts", bufs=1))
            wwpool = outer.enter_context(tc.tile_pool(name="wwp", bufs=1))
            stpool = outer.enter_context(tc.tile_pool(name="stp", bufs=1))
            statpool = outer.enter_context(tc.tile_pool(name="stats", bufs=1))
            pbig = outer.enter_context(
                tc.tile_pool(name="pbig", bufs=1, space="PSUM"))
            psmall = outer.enter_context(
                tc.tile_pool(name="psmall", bufs=1, space="PSUM"))
            drampool = outer.enter_context(
                tc.tile_pool(name="dramp", bufs=1, space="DRAM"))
            xbpool = outer.enter_context(tc.tile_pool(name="xbp", bufs=1))

            mask_sb = cpool.tile([128, 128], BF16, name="mask_sb",
                                 tag="mask_sb")
            gb_sb = cpool.tile([128, 16], F32, name="gb_sb", tag="gb_sb")
            ww_all = wwpool.tile([128, NLC * C],
                                 F8 if USE_FP8_OUT else BF16,
                                 name="ww_all", tag="ww")
            # stT: [p, (lc, tok)] with tok TIME-major (tok = t*N + j)
            stT = stpool.tile([128, NLC * TOK],
                              F8 if USE_FP8_OUT else BF16,
                              name="stT", tag="stT")

            stat_sum = statpool.tile([128, 32], F32, name="stat_sum",
                                     tag="stat_sum")
            stat_sq = statpool.tile([128, 32], F32, name="stat_sq",
                                    tag="stat_sq")
            red_in = statpool.tile([128, 16], F32, name="red_in", tag="red_in")
            red_out = statpool.tile([128, 16], F32, name="red_out",
                                    tag="red_out")
            scalev = statpool.tile([128, 8], F32, name="scalev", tag="scalev")
            biasv = statpool.tile([128, 8], F32, name="biasv", tag="biasv")

            cc_in = drampool.tile([128, 16], F32, name="cc_in", tag="cc_in")
            cc_out = drampool.tile([128, 16], F32, name="cc_out", tag="cc_out")
            cc_warm_in = drampool.tile([128, 1], F32, name="cc_warm_in",
                                       tag="cc_warm_in")
            cc_warm_out = drampool.tile([128, 1], F32, name="cc_warm_out",
                                        tag="cc_warm_out")
            # DRAM bounce buffer for the g repack: rows = TIME-major tokens
            g_dram = drampool.tile([TOK, L], BF16, name="g_dram",
                                   tag="g_dram")

            with ExitStack() as mid:
                thpool = mid.enter_context(tc.tile_pool(name="thp", bufs=1))
                gpool = mid.enter_context(tc.tile_pool(name="gp", bufs=1))
                attnpool = mid.enter_context(tc.tile_pool(name="attn", bufs=1))
                wg_all = gpool.tile([128, NCC * L], XDT, name="wg_all",
                                    tag="wg")

                # thT/phT: [p (l within lc), tok] ACTOR-major
                thT = [thpool.tile([128, TOK], BF16, name=f"thT{lc}",
                                   tag=f"thT{lc}") for lc in range(NLC)]
                phT = [thpool.tile([128, TOK], BF16, name=f"phT{lc}",
                                   tag=f"phT{lc}") for lc in range(NLC)]
                # g_act: one tile [128, NGRP*L]; group jg at cols jg*L..,
                # partitions = group tokens (jl, t): p = jl*T + t
                g_act = gpool.tile([128, NGRP * L], BF16, name="gact",
                                   tag="gact")
                # g_sp[i]: partitions = actors at time i (gathered via DMA)
                g_sp = [gpool.tile([128, L], BF16, name=f"gsp{i}",
                                   tag=f"gsp{i}") for i in range(T)]

                def sp_view(tile_ap, i):
                    """[128, TOK] actor-major tile -> time-i slice
                    (128 actors, stride T)."""
                    return tile_ap.rearrange("p (j t) -> p t j", t=T)[
                        :, i:i + 1, :]

                with ExitStack() as phase_a:
                    wpool = phase_a.enter_context(
                        tc.tile_pool(name="wp", bufs=1))

                    xbf = xbpool.tile([128, NTC * NCC * 512], XDT,
                                      name="xbf", tag="xbf")
                    xres = xbpool.tile([128, NCC * TOK], BF16,
                                       name="xres", tag="xres")
                    wt_all = wpool.tile([128, NCC * L], XDT, name="wt_all",
                                        tag="wt")
                    wp_all = wpool.tile([128, NCC * L],
                                        BF16 if PH_BF16 else XDT,
                                        name="wp_all", tag="wp")

                    XC = NCC * 512   # cols per token chunk

                    def xdst(tck, clo, chi):
                        # SBUF layout (tck, c, k) matches DRAM: contiguous
                        return xbf[:, tck * XC + clo * 512:
                                   tck * XC + chi * 512]

                    def xsrc(tck, clo, chi):
                        return xbf_d[:, tck * XC + clo * 512:
                                     tck * XC + chi * 512]

                    # gpsimd carries ONLY the warm-up collective: it
                    # blocks its queue until the cross-core rendezvous, so
                    # no loads may sit behind it
                    if USE_COLLECTIVE:
                        nc.gpsimd.dma_start(cc_warm_in[:], gb_d[:, 0:1])
                        nc.gpsimd.collective_compute(
                            "AllReduce", OP.add,
                            replica_groups=[list(range(N_CORES))],
                            ins=[cc_warm_in.opt()], outs=[cc_warm_out.opt()])
                    # halve the first wt/x transfers so the first matmuls
                    # start as early as possible
                    XRC = NCC * 512   # bf16 elems per xres chunk

                    def xresdst(tck):
                        return xres.rearrange("p (c k) -> p c k", c=NCC)[
                            :, :, ts(tck, 512)]

                    nc.sync.dma_start(wt_all[:, :NCC * L // 2],
                                      wt_d[:, :NCC * L // 2])
                    nc.scalar.dma_start(wt_all[:, NCC * L // 2:],
                                        wt_d[:, NCC * L // 2:])
                    nc.sync.dma_start(xdst(0, 0, 8), xsrc(0, 0, 8))
                    nc.scalar.dma_start(wp_all[:, :NCC * L // 2],
                                        wp_d[:, :NCC * L // 2])
                    nc.scalar.dma_start(wp_all[:, NCC * L // 2:],
                                        wp_d[:, NCC * L // 2:])
                    nc.sync.dma_start(xresdst(0), xres_d[:, ts(0, XRC)])
                    nc.sync.dma_start(wg_all[:], wg_d[:])
                    nc.scalar.dma_start(xresdst(1), xres_d[:, ts(1, XRC)])
                    nc.sync.dma_start(xdst(1, 0, 8), xsrc(1, 0, 8))
                    nc.scalar.dma_start(mask_sb[:], mask_d[:])
                    nc.scalar.dma_start(gb_sb[:], gb_d[:])
                    nc.sync.dma_start(xdst(2, 0, 8), xsrc(2, 0, 8))
                    nc.scalar.dma_start(xresdst(2), xres_d[:, ts(2, XRC)])
                    nc.sync.dma_start(xdst(3, 0, 8), xsrc(3, 0, 8))
                    nc.scalar.dma_start(xresdst(3), xres_d[:, ts(3, XRC)])
                    nc.scalar.dma_start(ww_all[:], ww_d[:])

                    def xsl(tck, c, off=0, n=512):
                        base = (tck * NCC + c) * 512
                        return xbf[:, base + off:base + off + n]

                    def wsl(w_all, c, lc):
                        return w_all[:, c * L + lc * 128:c * L + (lc + 1) * 128]

                    # temporal attention; INITIALIZES stT with a strided
                    # scatter into the time-major layout (split ACT/DVE)
                    pend_tp = []   # (jg, twp)

                    def emit_tw(jg):
                        twp = psmall.tile([128, 128], F32, name="ps_tw",
                                          tag="ps_small", bufs=4)
                        for lc in range(NLC):
                            nc.tensor.matmul(twp[:], phT[lc][:, ts(jg, 128)],
                                             thT[lc][:, ts(jg, 128)],
                                             start=(lc == 0),
                                             stop=(lc == NLC - 1))
                        pend_tp.append((jg, twp))

                    def emit_tp():
                        jg, twp = pend_tp.pop(0)
                        sb = attnpool.tile([128, 128], BF16, name="sb",
                                           tag="sb", bufs=3)
                        nc.vector.tensor_mul(sb[:], twp[:], mask_sb[:])
                        pp = psmall.tile([128, 512], F32, name="ps_tp",
                                         tag="ps_small", bufs=4)
                        for lc in range(NLC):
                            nc.tensor.matmul(pp[:, ts(lc, 128)],
                                             g_act[:, jg * L + lc * 128:
                                                   jg * L + (lc + 1) * 128],
                                             sb[:])
                        # pp cols (jl, t); scatter to time-major stT:
                        # dst col for (jl, t) = t*N + jg*JG + jl
                        dst = stT.rearrange("p (a t jg jl) -> p a jl t jg",
                                            a=NLC, t=T, jl=JG)[
                            :, :, :, :, jg]
                        src = pp.rearrange("p (a jl t) -> p a jl t",
                                           a=NLC, jl=JG)
                        tp_eff = TP_SCALE * (ATT_DESCALE if USE_FP8
                                             else 1.0)
                        nc.scalar.mul(dst, src, tp_eff)

                    # ------- phase 1: projections + g_act + temporal -------
                    xpair4 = xbf.rearrange("p (a c k) -> p a c k",
                                           a=NTC, c=NCC)

                    def xres_sl(tck, c):
                        base = c * TOK + tck * 512
                        return xres[:, base:base + 512]

                    for tck in range(NTC):
                        for (w_all, dst) in ((wt_all, thT), (wp_all, phT)):
                            is_ph = w_all is wp_all
                            use8 = USE_FP8 and not (PH_BF16 and is_ph)
                            wpair = w_all.rearrange("p (c l) -> p c l",
                                                    c=NCC)
                            for lc in range(NLC):
                                ps = pbig.tile([128, 512], F32, name="ps_proj",
                                               tag="ps_big", bufs=4)
                                if use8:
                                    for ci in range(NCC // 2):
                                        nc.tensor.matmul(
                                            ps[:],
                                            wpair[:, 2 * ci:2 * ci + 2,
                                                  lc * 128:(lc + 1) * 128],
                                            xpair4[:, tck,
                                                   2 * ci:2 * ci + 2, :],
                                            start=(ci == 0),
                                            stop=(ci == NCC // 2 - 1),
                                            perf_mode=DR)
                                else:
                                    for c in range(NCC):
                                        nc.tensor.matmul(
                                            ps[:], wsl(w_all, c, lc),
                                            xres_sl(tck, c)
                                            if (PH_BF16 and is_ph)
                                            else xsl(tck, c),
                                            start=(c == 0),
                                            stop=(c == NCC - 1))
                                nc.vector.tensor_copy(
                                    dst[lc][:, ts(tck, 512)], ps[:])
                        wgpair = wg_all.rearrange("p (c l) -> p c l",
                                                  c=NCC)
                        for jg in range(4 * tck, 4 * tck + 4):
                            ps = pbig.tile([128, 512], F32, name="ps_ga",
                                           tag="ps_big", bufs=4)
                            if USE_FP8:
                                for ci in range(NCC // 2):
                                    nc.tensor.matmul(
                                        ps[:],
                                        xpair4[:, jg // 4,
                                               2 * ci:2 * ci + 2,
                                               (jg % 4) * 128:
                                               (jg % 4 + 1) * 128],
                                        wgpair[:, 2 * ci:2 * ci + 2, :],
                                        start=(ci == 0),
                                        stop=(ci == NCC // 2 - 1),
                                        perf_mode=DR)
                            else:
                                for c in range(NCC):
                                    nc.tensor.matmul(
                                        ps[:], xsl(jg // 4, c,
                                                   (jg % 4) * 128, 128),
                                        wg_all[:, ts(c, 512)],
                                        start=(c == 0), stop=(c == NCC - 1))
                            nc.vector.tensor_copy(g_act[:, ts(jg, 512)],
                                                  ps[:])
                            emit_tw(jg)
                            if len(pend_tp) >= 2:
                                emit_tp()
                    while pend_tp:
                        emit_tp()

                # ------- phase 1.5: build g_sp[i] -------
                if USE_GATHER:
                    # two-hop repack through DRAM with plain access
                    # patterns: token (t, actor jg*8+jl) sits in g_act
                    # group-block jg at partition jl*T + t.
                    # hop 1 (per jl): SBUF partitions jl*T..jl*T+T ->
                    # DRAM rows (t, jg, jl)
                    dview = g_dram.rearrange("(t jg jl) l -> t jg jl l",
                                             t=T, jg=NGRP)
                    for jl in range(JG):
                        src = g_act.rearrange("p (jg l) -> p jg l",
                                              jg=NGRP)[jl * T:(jl + 1) * T]
                        eng = nc.sync if jl % 2 == 0 else nc.scalar
                        eng.dma_start(dview[:, :, jl, :], src)
                    # hop 2 (per i): contiguous DRAM block -> g_sp[i]
                    d2 = g_dram.rearrange("(t r) l -> t r l", t=T)
                    for i in range(T):
                        eng = nc.sync if i % 2 == 0 else nc.scalar
                        eng.dma_start(g_sp[i][:], d2[i])
                else:
                    # recompute g at each timestep from x (stride-T slices)
                    for i in range(T):
                        ps = pbig.tile([128, 512], F32, name="ps_g",
                                       tag="ps_big", bufs=4)
                        for c in range(NCC):
                            xc = xbf.rearrange(
                                "p (c j t) -> p c t j", c=NCC, t=T)[
                                :, c, i:i + 1, :]
                            nc.tensor.matmul(
                                ps[:], xc, wg_all[:, ts(c, 512)],
                                start=(c == 0), stop=(c == NCC - 1))
                        nc.scalar.copy(g_sp[i][:], ps[:])

                # ------- phase 2: spatial attention + out-projection -------
                with tc.tile_pool(name="outp", bufs=1) as outpool, \
                     tc.tile_pool(name="yp", bufs=1) as ypool, \
                     tc.tile_pool(name="sqp", bufs=1) as sqpool:
                    out_sb = []
                    inv_n = 1.0 / float(NTOK_GLOBAL)
                    for ct in range(NCC):
                        out_sb.append(outpool.tile(
                            [128, TOK], BF16, name=f"out{ct}", tag=f"out{ct}"))

                    swb = []
                    # all sw matmuls first: covers the g_sp gather DMAs
                    for i in range(T):
                        swp = psmall.tile([128, 128], F32, name="ps_sw",
                                          tag="ps_small", bufs=4)
                        for lc in range(NLC):
                            nc.tensor.matmul(swp[:], sp_view(phT[lc], i),
                                             sp_view(thT[lc], i),
                                             start=(lc == 0),
                                             stop=(lc == NLC - 1))
                        b = attnpool.tile([128, 128], BF16, name=f"swb{i}",
                                          tag=f"swb{i}", bufs=1)
                        nc.vector.tensor_copy(b[:], swp[:])
                        swb.append(b)

                    def emit_sp(i):
                        pp = psmall.tile([128, 512], F32, name="ps_sp",
                                         tag="ps_small", bufs=4)
                        for lc in range(NLC):
                            nc.tensor.matmul(pp[:, ts(lc, 128)],
                                             g_sp[i][:, ts(lc, 128)],
                                             swb[i][:])
                        # contiguous read-modify-write add into stT (time-
                        # major: time-i slice is cols i*128..(i+1)*128)
                        dst = stT.rearrange("p (a k) -> p a k", a=NLC)[
                            :, :, ts(i, 128)]
                        src = pp.rearrange("p (a k) -> p a k", a=NLC)
                        nc.vector.scalar_tensor_tensor(
                            out=dst, in0=src,
                            scalar=SP_SCALE * (ATT_DESCALE if USE_FP8
                                               else 1.0),
                            in1=dst, op0=OP.mult, op1=OP.add)

                    def x_res(tck, ct):
                        """x cols for TIME-chunk tck (t in 4tck..4tck+4,
                        all j), channel chunk ct, in (t, j) order.

                        xbf col for (j, t): ct*TOK + j*T + t.
                        """
                        v = xres.rearrange("p (c j t) -> p c t j",
                                           c=NCC, t=T)
                        return v[:, ct, 4 * tck:4 * tck + 4, :]

                    def emit_outproj_chunk(tck, ct):
                        o = out_sb[ct]
                        ps = pbig.tile([128, 512], F32, name="ps_out",
                                       tag="ps_big", bufs=4)
                        if USE_FP8_OUT:
                            wwp = ww_all.rearrange("p (a c1) -> p a c1",
                                                   a=NLC)
                            stp = stT.rearrange("p (a k) -> p a k", a=NLC)
                            for j in range(NLC // 2):
                                nc.tensor.matmul(
                                    ps[:],
                                    wwp[:, 2 * j:2 * j + 2,
                                        ct * 128:(ct + 1) * 128],
                                    stp[:, 2 * j:2 * j + 2,
                                        tck * 512:(tck + 1) * 512],
                                    start=(j == 0),
                                    stop=(j == NLC // 2 - 1),
                                    perf_mode=DR)
                        else:
                            for lc in range(NLC):
                                nc.tensor.matmul(
                                    ps[:],
                                    ww_all[:, lc * C + ct * 128:
                                           lc * C + (ct + 1) * 128],
                                    stT[:, lc * TOK + tck * 512:
                                        lc * TOK + tck * 512 + 512],
                                    start=(lc == 0), stop=(lc == NLC - 1))
                        col = ct * NTC + tck
                        nc.vector.scalar_tensor_tensor(
                            out=o[:, ts(tck, 512)], in0=ps[:],
                            scalar=1.0 / (ST_SCALE * W_SCALE)
                            if USE_FP8_OUT else 1.0,
                            in1=x_res(tck, ct),
                            op0=OP.mult, op1=OP.add,
                            accum_out=stat_sum[:, col:col + 1])
                        sq = sqpool.tile([128, 512], F32, name="sqscr",
                                         tag="sq", bufs=3)
                        nc.scalar.activation(
                            sq[:], o[:, ts(tck, 512)], ACT_FN.Square,
                            accum_out=stat_sq[:, col:col + 1])

                    # pipeline: spatial applies feed out-proj chunk by chunk;
                    # last chunk is ct-ordered so the stats collective can
                    # start before ct7 finishes
                    for i in range(4):
                        emit_sp(i)
                    for tck in range(NTC - 1):
                        for ct in range(NCC):
                            emit_outproj_chunk(tck, ct)
                            if ct < 4:
                                nxt = (tck + 1) * 4 + ct
                                if nxt < T:
                                    emit_sp(nxt)

                    def emit_stats_cc():
                        """AllReduce sum+sumsq for all channel tiles."""
                        nc.vector.tensor_reduce(
                            red_in[:, 0:8],
                            stat_sum.rearrange("p (a b) -> p a b", a=8),
                            axis=AX.X, op=OP.add)
                        nc.vector.tensor_reduce(
                            red_in[:, 8:16],
                            stat_sq.rearrange("p (a b) -> p a b", a=8),
                            axis=AX.X, op=OP.add)
                        if USE_COLLECTIVE:
                            nc.gpsimd.dma_start(cc_in[:], red_in[:])
                            nc.gpsimd.collective_compute(
                                "AllReduce", OP.add,
                                replica_groups=[list(range(N_CORES))],
                                ins=[cc_in.opt()], outs=[cc_out.opt()])
                            nc.gpsimd.dma_start(red_out[:], cc_out[:])
                        else:
                            nc.vector.tensor_scalar_mul(
                                red_out[:], red_in[:], float(N_CORES))

                    def emit_bn_params(part, lo, hi):
                        n = hi - lo
                        mean = statpool.tile([128, n], F32, name=f"mean{part}",
                                             tag=f"mean{part}")
                        var = statpool.tile([128, n], F32, name=f"var{part}",
                                            tag=f"var{part}")
                        std = statpool.tile([128, n], F32, name=f"std{part}",
                                            tag=f"std{part}")
                        rstd = statpool.tile([128, n], F32, name=f"rstd{part}",
                                             tag=f"rstd{part}")
                        nc.vector.tensor_scalar_mul(mean[:],
                                                    red_out[:, lo:hi], inv_n)
                        nc.vector.tensor_scalar_mul(
                            var[:], red_out[:, 8 + lo:8 + hi], inv_n)
                        nc.vector.tensor_mul(std[:], mean[:], mean[:])
                        nc.vector.tensor_tensor(var[:], var[:], std[:],
                                                op=OP.subtract)
                        nc.vector.tensor_scalar_add(var[:], var[:], BN_EPS)
                        nc.scalar.activation(std[:], var[:], ACT_FN.Sqrt,
                                             bias=0.0)
                        nc.vector.reciprocal(rstd[:], std[:])
                        nc.vector.tensor_mul(scalev[:, lo:hi], rstd[:],
                                             gb_sb[:, lo:hi])
                        nc.vector.tensor_mul(rstd[:], mean[:],
                                             scalev[:, lo:hi])
                        nc.vector.tensor_tensor(biasv[:, lo:hi],
                                                gb_sb[:, 8 + lo:8 + hi],
                                                rstd[:], op=OP.subtract)

                    def emit_apply(ct):
                        # DVE applies (2x bf16 mode, ~0.8us/tile) + ONE ACT
                        # tile; stores round-robin on sync/scalar/gpsimd
                        src = out_sb[ct][:]
                        dst = outy_d[:, ct * TOK:(ct + 1) * TOK]
                        if ct in (3, 7):
                            y = ypool.tile([128, TOK], BF16, name="ya",
                                           tag="ya", bufs=2)
                            nc.scalar.activation(
                                y[:], src, ACT_FN.Identity,
                                scale=scalev[:, ct:ct + 1],
                                bias=biasv[:, ct:ct + 1])
                        else:
                            y = ypool.tile([128, TOK], BF16, name="yb",
                                           tag="yb", bufs=4)
                            nc.vector.tensor_scalar(
                                out=y[:], in0=src,
                                scalar1=scalev[:, ct:ct + 1],
                                scalar2=biasv[:, ct:ct + 1],
                                op0=OP.mult, op1=OP.add)
                        eng = (nc.sync, nc.scalar, nc.gpsimd)[ct % 3]
                        eng.dma_start(dst, y[:])

                    # last token chunk; single collective after all stats
                    for ct in range(NCC):
                        emit_outproj_chunk(NTC - 1, ct)
                    emit_stats_cc()
                    emit_bn_params(0, 0, NCC)
                    for ct in range(NCC):
                        emit_apply(ct)

    nc.compile()
    return nc


def _get_compiled():
    global _compiled
    if _compiled is None:
        _compiled = _build()
    return _compiled


def _tile_rows(a, nchunk):
    """[R, X] -> [128, nchunk*X] with row p, col (c*X+x) = a[c*128+p, x]."""
    R, X = a.shape
    assert R == nchunk * 128
    return np.ascontiguousarray(
        a.reshape(nchunk, 128, X).transpose(1, 0, 2).reshape(128, -1))


def kernel(x, Wt, Wp, Wg, Ww, gamma, beta, _trace=False, _trace_kwargs=None):
    global _last_results
    nc = _get_compiled()

    x = np.asarray(x, dtype=np.float32)
    Wt = np.asarray(Wt, dtype=np.float32)
    Wp = np.asarray(Wp, dtype=np.float32)
    Wg = np.asarray(Wg, dtype=np.float32)
    Ww = np.asarray(Ww, dtype=np.float32)
    gamma = np.asarray(gamma, dtype=np.float32)
    beta = np.asarray(beta, dtype=np.float32)

    bf = ml_dtypes.bfloat16
    xdt = ml_dtypes.float8_e4m3fn if USE_FP8 else bf
    wmul = W_SCALE if USE_FP8 else 1.0
    wt_t = _tile_rows(np.ascontiguousarray(Wt.T) * wmul, NCC).astype(xdt)
    wp_t = _tile_rows(np.ascontiguousarray(Wp.T)
                      * (1.0 if PH_BF16 else wmul),
                      NCC).astype(bf if PH_BF16 else xdt)
    wg_t = _tile_rows(np.ascontiguousarray(Wg.T) * wmul, NCC).astype(xdt)
    owdt = ml_dtypes.float8_e4m3fn if USE_FP8_OUT else bf
    ww_t = _tile_rows(np.ascontiguousarray(Ww.T)
                      * (W_SCALE if USE_FP8_OUT else 1.0),
                      NLC).astype(owdt)                            # [L, C]
    r = np.arange(128)
    mask = (r[:, None] // T == r[None, :] // T).astype(bf)
    gb = np.concatenate(
        [gamma.reshape(NCC, 128).T,
         beta.reshape(NCC, 128).T], axis=1).astype(np.float32)  # [128, 16]

    # actor-major token order (tok = j*T + t), cols laid out (tck, c, k)
    xa = x.transpose(0, 2, 1, 3).reshape(B, TOK, C)
    in_maps = []
    for b in range(B):
        xT = np.ascontiguousarray(xa[b].T)            # [C, TOK] f32
        xt = xT.reshape(NCC, 128, NTC, 512).transpose(1, 2, 0, 3)
        xt = np.ascontiguousarray(xt.reshape(128, -1))  # [128,(tck,c,k)]
        in_maps.append(dict(
            xbf=xt.astype(xdt), xres=xt.astype(bf),
            wt=wt_t, wp=wp_t, wg=wg_t, ww=ww_t,
            mask=mask, gb=gb))

    res = run_bass_kernel_spmd(nc, in_maps, list(range(N_CORES)),
                               trace=_trace, **(_trace_kwargs or {}))
    _last_results = res

    ys = []
    for b in range(B):
        o = np.asarray(res.results[b]["outy"], dtype=np.float32)
        # [128, (ct, tok)] with tok TIME-major -> [TOK, C] -> [T, N, C]
        o = o.reshape(128, NCC, TOK).transpose(2, 1, 0).reshape(TOK, C)
        ys.append(o.reshape(T, N, C))
    return np.stack(ys)


# revision 31
# speedup vs baseline: 1.0536x; 1.0536x over previous
"""Trainium2 Bass kernel for CrossInferBlock (spatial+temporal cross attention
+ out-projection + residual + BatchNorm over (B,T,N)).

Sharding: data-parallel over B across 8 NeuronCores (one batch element per
core). BN batch statistics are all-reduced across cores (8KB collective).

All matmuls run in bf16 (fp32 PSUM accumulate); residual/stats/BN in fp32.

Token orders: x and the projections (thT/phT/g) use ACTOR-MAJOR order
(tok = j*T + t) so every matmul operand is a legal single-stride access
pattern: temporal groups (8 actors x 16 timesteps) are contiguous 128-token
slices, spatial slices (one timestep, all 128 actors) are stride-T slices.
stT and the output use TIME-MAJOR order (tok = t*N + j): the temporal apply
pays a strided scatter once per group (hidden under the long projection
phase, split across ACT and DVE), which makes the phase-2 spatial adds
contiguous AND lets the out-projection start per 512-token time-chunk as
soon as its 4 spatial slices have landed.

g is projected ONCE (actor-group tiles, stationary operand read straight
from x); the 16 per-timestep tiles needed by the spatial applies are derived
with partition-gather SBUF->SBUF DMAs instead of a second projection pass.

The BN stats collective is split 7/1 so the first AllReduce's rendezvous
overlaps the out-projection tail and the bulk BN apply overlaps the second;
BN apply+store is split across the ACT and DVE engines with bf16 stores on
two HWDGE rings (the host upcasts to fp32).

All DRAM tensors are pre-tiled host-side to [128, X] exactly matching their
SBUF destination so every load is a full-row contiguous DMA.
"""

import sys

if "/opt/trn_rl_repo" not in sys.path:
    sys.path.insert(0, "/opt/trn_rl_repo")

import numpy as np
import ml_dtypes

import concourse.bass as bass
import concourse.bacc as bacc
import concourse.tile as tile
import concourse.mybir as mybir
from concourse.bass_utils import run_bass_kernel_spmd
from contextlib import ExitStack

F32 = mybir.dt.float32
BF16 = mybir.dt.bfloat16
F8 = mybir.dt.float8e4
DR = mybir.MatmulPerfMode.DoubleRow
AX = mybir.AxisListType
OP = mybir.AluOpType
ACT_FN = mybir.ActivationFunctionType

N_CORES = 8
B, T, N, C = 8, 16, 128, 1024
L = C // 2            # 512
TOK = T * N           # 2048 tokens per batch element
NTOK_GLOBAL = B * T * N
JG = 8                # actors per temporal group
NGRP = N // JG        # 16 groups
BN_EPS = 1e-5

SP_SCALE = 1.0 / (N * (T + N))   # spatial: /N then /(T+N)
TP_SCALE = 1.0 / (T * (T + N))   # temporal: /T then /(T+N)

NCC = C // 128     # 8 c-chunks
NLC = L // 128     # 4 l-chunks
NTC = TOK // 512   # 4 token chunks
CT_SPLIT = 7       # channel tiles covered by the first stats collective

_compiled = None
_last_results = None

USE_COLLECTIVE = True
USE_GATHER = True        # g_sp via SBUF->SBUF partition-gather DMA
USE_FP8 = True           # fp8e4 DoubleRow for theta/g projections
PH_BF16 = False          # phi stays bf16 (recovers quantization margin)
USE_FP8_OUT = False      # fp8e4 DoubleRow out-projection (stT + Ww in fp8)
W_SCALE = 16.0           # host premultiplies Wt/Wg (avoids fp8 subnormals)
ST_SCALE = 16.0 if USE_FP8_OUT else 1.0   # stT stored pre-scaled in fp8
# th x16, ph x16 (or x1 if PH_BF16), g x16 -> tw/sw and tp/sp carry the
# product; stT absorbs ST_SCALE
_PROJ = W_SCALE * W_SCALE * (1.0 if PH_BF16 else W_SCALE)
ATT_DESCALE = ST_SCALE / _PROJ



def ts(i, size):
    return bass.ts(i, size)


def _build():
    nc = bacc.Bacc("TRN2", target_bir_lowering=False, debug=False,
                   num_devices=N_CORES)

    # ---- DRAM I/O (pre-tiled [128, X]) ----
    # xbf rows: partition p; cols (tck, c, k): actor-major tokens,
    # x[c*128+p, tck*512+k] with tok = j*T + t
    XDT = F8 if USE_FP8 else BF16
    xbf_d = nc.dram_tensor("xbf", [128, NTC * NCC * 512], XDT,
                           kind="ExternalInput")
    xres_d = nc.dram_tensor("xres", [128, NTC * NCC * 512], BF16,
                            kind="ExternalInput")
    wt_d = nc.dram_tensor("wt", [128, NCC * L], XDT, kind="ExternalInput")
    wp_d = nc.dram_tensor("wp", [128, NCC * L],
                          BF16 if PH_BF16 else XDT, kind="ExternalInput")
    wg_d = nc.dram_tensor("wg", [128, NCC * L], XDT, kind="ExternalInput")
    ww_d = nc.dram_tensor("ww", [128, NLC * C],
                          F8 if USE_FP8_OUT else BF16, kind="ExternalInput")
    mask_d = nc.dram_tensor("mask", [128, 128], BF16, kind="ExternalInput")
    gb_d = nc.dram_tensor("gb", [128, 16], F32, kind="ExternalInput")
    # outy rows: partition p; cols (ct, tok): TIME-major tokens
    outy_d = nc.dram_tensor("outy", [128, NCC * TOK], BF16,
                            kind="ExternalOutput")

    with tile.TileContext(nc) as tc:
        with ExitStack() as outer:
            # ---------------- persistent pools ----------------
            cpool = outer.enter_context(tc.tile_pool(name="consts", bufs=1))
            wwpool = outer.enter_context(tc.tile_pool(name="wwp", bufs=1))
            stpool = outer.enter_context(tc.tile_pool(name="stp", bufs=1))
            statpool = outer.enter_context(tc.tile_pool(name="stats", bufs=1))
            pbig = outer.enter_context(
                tc.tile_pool(name="pbig", bufs=1, space="PSUM"))
            psmall = outer.enter_context(
                tc.tile_pool(name="psmall", bufs=1, space="PSUM"))
            drampool = outer.enter_context(
                tc.tile_pool(name="dramp", bufs=1, space="DRAM"))
            xbpool = outer.enter_context(tc.tile_pool(name="xbp", bufs=1))

            mask_sb = cpool.tile([128, 128], BF16, name="mask_sb",
                                 tag="mask_sb")
            gb_sb = cpool.tile([128, 16], F32, name="gb_sb", tag="gb_sb")
            ww_all = wwpool.tile([128, NLC * C],
                                 F8 if USE_FP8_OUT else BF16,
                                 name="ww_all", tag="ww")
            # stT: [p, (lc, tok)] with tok TIME-major (tok = t*N + j)
            stT = stpool.tile([128, NLC * TOK],
                              F8 if USE_FP8_OUT else BF16,
                              name="stT", tag="stT")

            stat_sum = statpool.tile([128, 32], F32, name="stat_sum",
                                     tag="stat_sum")
            stat_sq = statpool.tile([128, 32], F32, name="stat_sq",
                                    tag="stat_sq")
            red_in = statpool.tile([128, 16], F32, name="red_in", tag="red_in")
            red_out = statpool.tile([128, 16], F32, name="red_out",
                                    tag="red_out")
            scalev = statpool.tile([128, 8], F32, name="scalev", tag="scalev")
            biasv = statpool.tile([128, 8], F32, name="biasv", tag="biasv")

            cc_in = drampool.tile([128, 16], F32, name="cc_in", tag="cc_in")
            cc_out = drampool.tile([128, 16], F32, name="cc_out", tag="cc_out")
            cc_warm_in = drampool.tile([128, 1], F32, name="cc_warm_in",
                                       tag="cc_warm_in")
            cc_warm_out = drampool.tile([128, 1], F32, name="cc_warm_out",
                                        tag="cc_warm_out")
            # DRAM bounce buffer for the g repack: rows = TIME-major tokens
            g_dram = drampool.tile([TOK, L], BF16, name="g_dram",
                                   tag="g_dram")

            with ExitStack() as mid:
                thpool = mid.enter_context(tc.tile_pool(name="thp", bufs=1))
                gpool = mid.enter_context(tc.tile_pool(name="gp", bufs=1))
                attnpool = mid.enter_context(tc.tile_pool(name="attn", bufs=1))
                wg_all = gpool.tile([128, NCC * L], XDT, name="wg_all",
                                    tag="wg")

                # thT/phT: [p (l within lc), tok] ACTOR-major
                thT = [thpool.tile([128, TOK], BF16, name=f"thT{lc}",
                                   tag=f"thT{lc}") for lc in range(NLC)]
                phT = [thpool.tile([128, TOK], BF16, name=f"phT{lc}",
                                   tag=f"phT{lc}") for lc in range(NLC)]
                # g_act: one tile [128, NGRP*L]; group jg at cols jg*L..,
                # partitions = group tokens (jl, t): p = jl*T + t
                g_act = gpool.tile([128, NGRP * L], BF16, name="gact",
                                   tag="gact")
                # g_sp[i]: partitions = actors at time i (gathered via DMA)
                g_sp = [gpool.tile([128, L], BF16, name=f"gsp{i}",
                                   tag=f"gsp{i}") for i in range(T)]

                def sp_view(tile_ap, i):
                    """[128, TOK] actor-major tile -> time-i slice
                    (128 actors, stride T)."""
                    return tile_ap.rearrange("p (j t) -> p t j", t=T)[
                        :, i:i + 1, :]

                with ExitStack() as phase_a:
                    wpool = phase_a.enter_context(
                        tc.tile_pool(name="wp", bufs=1))

                    xbf = xbpool.tile([128, NTC * NCC * 512], XDT,
                                      name="xbf", tag="xbf")
                    xres = xbpool.tile([128, NCC * TOK], BF16,
                                       name="xres", tag="xres")
                    wt_all = wpool.tile([128, NCC * L], XDT, name="wt_all",
                                        tag="wt")
                    wp_all = wpool.tile([128, NCC * L],
                                        BF16 if PH_BF16 else XDT,
                                        name="wp_all", tag="wp")

                    XC = NCC * 512   # cols per token chunk

                    def xdst(tck, clo, chi):
                        # SBUF layout (tck, c, k) matches DRAM: contiguous
                        return xbf[:, tck * XC + clo * 512:
                                   tck * XC + chi * 512]

                    def xsrc(tck, clo, chi):
                        return xbf_d[:, tck * XC + clo * 512:
                                     tck * XC + chi * 512]

                    # gpsimd carries ONLY the warm-up collective: it
                    # blocks its queue until the cross-core rendezvous, so
                    # no loads may sit behind it
                    if USE_COLLECTIVE:
                        nc.gpsimd.dma_start(cc_warm_in[:], gb_d[:, 0:1])
                        nc.gpsimd.collective_compute(
                            "AllReduce", OP.add,
                            replica_groups=[list(range(N_CORES))],
                            ins=[cc_warm_in.opt()], outs=[cc_warm_out.opt()])
                    # halve the first wt/x transfers so the first matmuls
                    # start as early as possible
                    XRC = NCC * 512   # bf16 elems per xres chunk

                    def xresdst(tck):
                        return xres.rearrange("p (c k) -> p c k", c=NCC)[
                            :, :, ts(tck, 512)]

                    nc.sync.dma_start(wt_all[:, :NCC * L // 2],
                                      wt_d[:, :NCC * L // 2])
                    nc.scalar.dma_start(wt_all[:, NCC * L // 2:],
                                        wt_d[:, NCC * L // 2:])
                    nc.sync.dma_start(xdst(0, 0, 8), xsrc(0, 0, 8))
                    nc.scalar.dma_start(wp_all[:, :NCC * L // 2],
                                        wp_d[:, :NCC * L // 2])
                    nc.scalar.dma_start(wp_all[:, NCC * L // 2:],
                                        wp_d[:, NCC * L // 2:])
                    nc.sync.dma_start(xresdst(0), xres_d[:, ts(0, XRC)])
                    nc.sync.dma_start(wg_all[:], wg_d[:])
                    nc.scalar.dma_start(xresdst(1), xres_d[:, ts(1, XRC)])
                    nc.sync.dma_start(xdst(1, 0, 8), xsrc(1, 0, 8))
                    nc.scalar.dma_start(mask_sb[:], mask_d[:])
                    nc.scalar.dma_start(gb_sb[:], gb_d[:])
                    nc.sync.dma_start(xdst(2, 0, 8), xsrc(2, 0, 8))
                    nc.scalar.dma_start(xresdst(2), xres_d[:, ts(2, XRC)])
                    nc.sync.dma_start(xdst(3, 0, 8), xsrc(3, 0, 8))
                    nc.scalar.dma_start(xresdst(3), xres_d[:, ts(3, XRC)])
                    nc.scalar.dma_start(ww_all[:], ww_d[:])

                    def xsl(tck, c, off=0, n=512):
                        base = (tck * NCC + c) * 512
                        return xbf[:, base + off:base + off + n]

                    def wsl(w_all, c, lc):
                        return w_all[:, c * L + lc * 128:c * L + (lc + 1) * 128]

                    # temporal attention; INITIALIZES stT with a strided
                    # scatter into the time-major layout (split ACT/DVE)
                    pend_tp = []   # (jg, twp)

                    def emit_tw(jg):
                        twp = psmall.tile([128, 128], F32, name="ps_tw",
                                          tag="ps_small", bufs=4)
                        for lc in range(NLC):
                            nc.tensor.matmul(twp[:], phT[lc][:, ts(jg, 128)],
                                             thT[lc][:, ts(jg, 128)],
                                             start=(lc == 0),
                                             stop=(lc == NLC - 1))
                        pend_tp.append((jg, twp))

                    def emit_tp():
                        jg, twp = pend_tp.pop(0)
                        sb = attnpool.tile([128, 128], BF16, name="sb",
                                           tag="sb", bufs=3)
                        nc.vector.tensor_mul(sb[:], twp[:], mask_sb[:])
                        pp = psmall.tile([128, 512], F32, name="ps_tp",
                                         tag="ps_small", bufs=4)
                        for lc in range(NLC):
                            nc.tensor.matmul(pp[:, ts(lc, 128)],
                                             g_act[:, jg * L + lc * 128:
                                                   jg * L + (lc + 1) * 128],
                                             sb[:])
                        # pp cols (jl, t); scatter to time-major stT:
                        # dst col for (jl, t) = t*N + jg*JG + jl
                        dst = stT.rearrange("p (a t jg jl) -> p a jl t jg",
                                            a=NLC, t=T, jl=JG)[
                            :, :, :, :, jg]
                        src = pp.rearrange("p (a jl t) -> p a jl t",
                                           a=NLC, jl=JG)
                        tp_eff = TP_SCALE * (ATT_DESCALE if USE_FP8
                                             else 1.0)
                        nc.scalar.mul(dst, src, tp_eff)

                    # ------- phase 1: projections + g_act + temporal -------
                    xpair4 = xbf.rearrange("p (a c k) -> p a c k",
                                           a=NTC, c=NCC)

                    def xres_sl(tck, c):
                        base = c * TOK + tck * 512
                        return xres[:, base:base + 512]

                    for tck in range(NTC):
                        for (w_all, dst) in ((wt_all, thT), (wp_all, phT)):
                            is_ph = w_all is wp_all
                            use8 = USE_FP8 and not (PH_BF16 and is_ph)
                            wpair = w_all.rearrange("p (c l) -> p c l",
                                                    c=NCC)
                            for lc in range(NLC):
                                ps = pbig.tile([128, 512], F32, name="ps_proj",
                                               tag="ps_big", bufs=4)
                                if use8:
                                    for ci in range(NCC // 2):
                                        nc.tensor.matmul(
                                            ps[:],
                                            wpair[:, 2 * ci:2 * ci + 2,
                                                  lc * 128:(lc + 1) * 128],
                                            xpair4[:, tck,
                                                   2 * ci:2 * ci + 2, :],
                                            start=(ci == 0),
                                            stop=(ci == NCC // 2 - 1),
                                            perf_mode=DR)
                                else:
                                    for c in range(NCC):
                                        nc.tensor.matmul(
                                            ps[:], wsl(w_all, c, lc),
                                            xres_sl(tck, c)
                                            if (PH_BF16 and is_ph)
                                            else xsl(tck, c),
                                            start=(c == 0),
                                            stop=(c == NCC - 1))
                                nc.vector.tensor_copy(
                                    dst[lc][:, ts(tck, 512)], ps[:])
                        wgpair = wg_all.rearrange("p (c l) -> p c l",
                                                  c=NCC)
                        for jg in range(4 * tck, 4 * tck + 4):
                            ps = pbig.tile([128, 512], F32, name="ps_ga",
                                           tag="ps_big", bufs=4)
                            if USE_FP8:
                                for ci in range(NCC // 2):
                                    nc.tensor.matmul(
                                        ps[:],
                                        xpair4[:, jg // 4,
                                               2 * ci:2 * ci + 2,
                                               (jg % 4) * 128:
                                               (jg % 4 + 1) * 128],
                                        wgpair[:, 2 * ci:2 * ci + 2, :],
                                        start=(ci == 0),
                                        stop=(ci == NCC // 2 - 1),
                                        perf_mode=DR)
                            else:
                                for c in range(NCC):
                                    nc.tensor.matmul(
                                        ps[:], xsl(jg // 4, c,
                                                   (jg % 4) * 128, 128),
                                        wg_all[:, ts(c, 512)],
                                        start=(c == 0), stop=(c == NCC - 1))
                            nc.vector.tensor_copy(g_act[:, ts(jg, 512)],
                                                  ps[:])
                            emit_tw(jg)
                            if len(pend_tp) >= 2:
                                emit_tp()
                    while pend_tp:
                        emit_tp()

                # ------- phase 1.5: build g_sp[i] -------
                if USE_GATHER:
                    # two-hop repack through DRAM with plain access
                    # patterns: token (t, actor jg*8+jl) sits in g_act
                    # group-block jg at partition jl*T + t.
                    # hop 1 (per jl): SBUF partitions jl*T..jl*T+T ->
                    # DRAM rows (t, jg, jl)
                    dview = g_dram.rearrange("(t jg jl) l -> t jg jl l",
                                             t=T, jg=NGRP)
                    for jl in range(JG):
                        src = g_act.rearrange("p (jg l) -> p jg l",
                                              jg=NGRP)[jl * T:(jl + 1) * T]
                        eng = nc.sync if jl % 2 == 0 else nc.scalar
                        eng.dma_start(dview[:, :, jl, :], src)
                    # hop 2 (per i): contiguous DRAM block -> g_sp[i]
                    d2 = g_dram.rearrange("(t r) l -> t r l", t=T)
                    for i in range(T):
                        eng = nc.sync if i % 2 == 0 else nc.scalar
                        eng.dma_start(g_sp[i][:], d2[i])
                else:
                    # recompute g at each timestep from x (stride-T slices)
                    for i in range(T):
                        ps = pbig.tile([128, 512], F32, name="ps_g",
                                       tag="ps_big", bufs=4)
                        for c in range(NCC):
                            xc = xbf.rearrange(
                                "p (c j t) -> p c t j", c=NCC, t=T)[
                                :, c, i:i + 1, :]
                            nc.tensor.matmul(
                                ps[:], xc, wg_all[:, ts(c, 512)],
                                start=(c == 0), stop=(c == NCC - 1))
                        nc.scalar.copy(g_sp[i][:], ps[:])

                # ------- phase 2: spatial attention + out-projection -------
                with tc.tile_pool(name="outp", bufs=1) as outpool, \
                     tc.tile_pool(name="yp", bufs=1) as ypool, \
                     tc.tile_pool(name="sqp", bufs=1) as sqpool:
                    out_sb = []
                    inv_n = 1.0 / float(NTOK_GLOBAL)
                    for ct in range(NCC):
                        out_sb.append(outpool.tile(
                            [128, TOK], BF16, name=f"out{ct}", tag=f"out{ct}"))

                    swb = []
                    # all sw matmuls first: covers the g_sp gather DMAs
                    for i in range(T):
                        swp = psmall.tile([128, 128], F32, name="ps_sw",
                                          tag="ps_small", bufs=4)
                        for lc in range(NLC):
                            nc.tensor.matmul(swp[:], sp_view(phT[lc], i),
                                             sp_view(thT[lc], i),
                                             start=(lc == 0),
                                             stop=(lc == NLC - 1))
                        b = attnpool.tile([128, 128], BF16, name=f"swb{i}",
                                          tag=f"swb{i}", bufs=1)
                        nc.vector.tensor_copy(b[:], swp[:])
                        swb.append(b)

                    def emit_sp(i):
                        pp = psmall.tile([128, 512], F32, name="ps_sp",
                                         tag="ps_small", bufs=4)
                        for lc in range(NLC):
                            nc.tensor.matmul(pp[:, ts(lc, 128)],
                                             g_sp[i][:, ts(lc, 128)],
                                             swb[i][:])
                        # contiguous read-modify-write add into stT (time-
                        # major: time-i slice is cols i*128..(i+1)*128)
                        dst = stT.rearrange("p (a k) -> p a k", a=NLC)[
                            :, :, ts(i, 128)]
                        src = pp.rearrange("p (a k) -> p a k", a=NLC)
                        nc.vector.scalar_tensor_tensor(
                            out=dst, in0=src,
                            scalar=SP_SCALE * (ATT_DESCALE if USE_FP8
                                               else 1.0),
                            in1=dst, op0=OP.mult, op1=OP.add)

                    def x_res(tck, ct):
                        """x cols for TIME-chunk tck (t in 4tck..4tck+4,
                        all j), channel chunk ct, in (t, j) order.

                        xbf col for (j, t): ct*TOK + j*T + t.
                        """
                        v = xres.rearrange("p (c j t) -> p c t j",
                                           c=NCC, t=T)
                        return v[:, ct, 4 * tck:4 * tck + 4, :]

                    def emit_outproj_chunk(tck, ct):
                        o = out_sb[ct]
                        ps = pbig.tile([128, 512], F32, name="ps_out",
                                       tag="ps_big", bufs=4)
                        if USE_FP8_OUT:
                            wwp = ww_all.rearrange("p (a c1) -> p a c1",
                                                   a=NLC)
                            stp = stT.rearrange("p (a k) -> p a k", a=NLC)
                            for j in range(NLC // 2):
                                nc.tensor.matmul(
                                    ps[:],
                                    wwp[:, 2 * j:2 * j + 2,
                                        ct * 128:(ct + 1) * 128],
                                    stp[:, 2 * j:2 * j + 2,
                                        tck * 512:(tck + 1) * 512],
                                    start=(j == 0),
                                    stop=(j == NLC // 2 - 1),
                                    perf_mode=DR)
                        else:
                            for lc in range(NLC):
                                nc.tensor.matmul(
                                    ps[:],
                                    ww_all[:, lc * C + ct * 128:
                                           lc * C + (ct + 1) * 128],
                                    stT[:, lc * TOK + tck * 512:
                                        lc * TOK + tck * 512 + 512],
                                    start=(lc == 0), stop=(lc == NLC - 1))
                        col = ct * NTC + tck
                        nc.vector.scalar_tensor_tensor(
                            out=o[:, ts(tck, 512)], in0=ps[:],
                            scalar=1.0 / (ST_SCALE * W_SCALE)
                            if USE_FP8_OUT else 1.0,
                            in1=x_res(tck, ct),
                            op0=OP.mult, op1=OP.add,
                            accum_out=stat_sum[:, col:col + 1])
                        sq = sqpool.tile([128, 512], F32, name="sqscr",
                                         tag="sq", bufs=3)
                        nc.scalar.activation(
                            sq[:], o[:, ts(tck, 512)], ACT_FN.Square,
                            accum_out=stat_sq[:, col:col + 1])

                    # pipeline: spatial applies feed out-proj chunk by chunk;
                    # last chunk is ct-ordered so the stats collective can
                    # start before ct7 finishes
                    for i in range(4):
                        emit_sp(i)
                    for tck in range(NTC - 1):
                        for ct in range(NCC):
                            emit_outproj_chunk(tck, ct)
                            if ct < 4:
                                nxt = (tck + 1) * 4 + ct
                                if nxt < T:
                                    emit_sp(nxt)

                    def emit_stats_cc():
                        """AllReduce sum+sumsq for all channel tiles."""
                        nc.vector.tensor_reduce(
                            red_in[:, 0:8],
                            stat_sum.rearrange("p (a b) -> p a b", a=8),
                            axis=AX.X, op=OP.add)
                        nc.vector.tensor_reduce(
                            red_in[:, 8:16],
                            stat_sq.rearrange("p (a b) -> p a b", a=8),
                            axis=AX.X, op=OP.add)
                        if USE_COLLECTIVE:
                            nc.gpsimd.dma_start(cc_in[:], red_in[:])
                            nc.gpsimd.collective_compute(
                                "AllReduce", OP.add,
                                replica_groups=[list(range(N_CORES))],
                                ins=[cc_in.opt()], outs=[cc_out.opt()])
                            nc.gpsimd.dma_start(red_out[:], cc_out[:])
                        else:
                            nc.vector.tensor_scalar_mul(
                                red_out[:], red_in[:], float(N_CORES))

                    def emit_bn_params(part, lo, hi):
                        n = hi - lo
                        mean = statpool.tile([128, n], F32, name=f"mean{part}",
                                             tag=f"mean{part}")
                        var = statpool.tile([128, n], F32, name=f"var{part}",
                                            tag=f"var{part}")
                        std = statpool.tile([128, n], F32, name=f"std{part}",
                                            tag=f"std{part}")
                        rstd = statpool.tile([128, n], F32, name=f"rstd{part}",
                                             tag=f"rstd{part}")
                        nc.vector.tensor_scalar_mul(mean[:],
                                                    red_out[:, lo:hi], inv_n)
                        nc.vector.tensor_scalar_mul(
                            var[:], red_out[:, 8 + lo:8 + hi], inv_n)
                        nc.vector.tensor_mul(std[:], mean[:], mean[:])
                        nc.vector.tensor_tensor(var[:], var[:], std[:],
                                                op=OP.subtract)
                        nc.vector.tensor_scalar_add(var[:], var[:], BN_EPS)
                        nc.scalar.activation(std[:], var[:], ACT_FN.Sqrt,
                                             bias=0.0)
                        nc.vector.reciprocal(rstd[:], std[:])
                        nc.vector.tensor_mul(scalev[:, lo:hi], rstd[:],
                                             gb_sb[:, lo:hi])
                        nc.vector.tensor_mul(rstd[:], mean[:],
                                             scalev[:, lo:hi])
                        nc.vector.tensor_tensor(biasv[:, lo:hi],
                                                gb_sb[:, 8 + lo:8 + hi],
                                                rstd[:], op=OP.subtract)

                    def emit_apply(ct):
                        # DVE applies (2x bf16 mode, ~0.8us/tile) + ONE ACT
                        # tile; stores round-robin on sync/scalar/gpsimd
                        src = out_sb[ct][:]
                        dst = outy_d[:, ct * TOK:(ct + 1) * TOK]
                        if ct in (3, 7):
                            y = ypool.tile([128, TOK], BF16, name="ya",
                                           tag="ya", bufs=2)
                            nc.scalar.activation(
                                y[:], src, ACT_FN.Identity,
                                scale=scalev[:, ct:ct + 1],
                                bias=biasv[:, ct:ct + 1])
                        else:
                            y = ypool.tile([128, TOK], BF16, name="yb",
                                           tag="yb", bufs=4)
                            nc.vector.tensor_scalar(
                                out=y[:], in0=src,
                                scalar1=scalev[:, ct:ct + 1],
                                scalar2=biasv[:, ct:ct + 1],
                                op0=OP.mult, op1=OP.add)
                        eng = (nc.sync, nc.scalar, nc.gpsimd)[ct % 3]
                        eng.dma_start(dst, y[:])

                    # last token chunk; single collective after all stats
                    for ct in range(NCC):
                        emit_outproj_chunk(NTC - 1, ct)
                    emit_stats_cc()
                    emit_bn_params(0, 0, NCC)
                    for ct in range(NCC):
                        emit_apply(ct)

    nc.compile()
    return nc


def _get_compiled():
    global _compiled
    if _compiled is None:
        _compiled = _build()
    return _compiled


def _tile_rows(a, nchunk):
    """[R, X] -> [128, nchunk*X] with row p, col (c*X+x) = a[c*128+p, x]."""
    R, X = a.shape
    assert R == nchunk * 128
    return np.ascontiguousarray(
        a.reshape(nchunk, 128, X).transpose(1, 0, 2).reshape(128, -1))


def kernel(x, Wt, Wp, Wg, Ww, gamma, beta, _trace=False, _trace_kwargs=None):
    global _last_results
    nc = _get_compiled()

    x = np.asarray(x, dtype=np.float32)
    Wt = np.asarray(Wt, dtype=np.float32)
    Wp = np.asarray(Wp, dtype=np.float32)
    Wg = np.asarray(Wg, dtype=np.float32)
    Ww = np.asarray(Ww, dtype=np.float32)
    gamma = np.asarray(gamma, dtype=np.float32)
    beta = np.asarray(beta, dtype=np.float32)

    bf = ml_dtypes.bfloat16
    xdt = ml_dtypes.float8_e4m3fn if USE_FP8 else bf
    wmul = W_SCALE if USE_FP8 else 1.0
    wt_t = _tile_rows(np.ascontiguousarray(Wt.T) * wmul, NCC).astype(xdt)
    wp_t = _tile_rows(np.ascontiguousarray(Wp.T)
                      * (1.0 if PH_BF16 else wmul),
                      NCC).astype(bf if PH_BF16 else xdt)
    wg_t = _tile_rows(np.ascontiguousarray(Wg.T) * wmul, NCC).astype(xdt)
    owdt = ml_dtypes.float8_e4m3fn if USE_FP8_OUT else bf
    ww_t = _tile_rows(np.ascontiguousarray(Ww.T)
                      * (W_SCALE if USE_FP8_OUT else 1.0),
                      NLC).astype(owdt)                            # [L, C]
    r = np.arange(128)
    mask = (r[:, None] // T == r[None, :] // T).astype(bf)
    gb = np.concatenate(
        [gamma.reshape(NCC, 128).T,
         beta.reshape(NCC, 128).T], axis=1).astype(np.float32)  # [128, 16]

    # actor-major token order (tok = j*T + t), cols laid out (tck, c, k)
    xa = x.transpose(0, 2, 1, 3).reshape(B, TOK, C)
    in_maps = []
    for b in range(B):
        xT = np.ascontiguousarray(xa[b].T)            # [C, TOK] f32
        xt = xT.reshape(NCC, 128, NTC, 512).transpose(1, 2, 0, 3)
        xt = np.ascontiguousarray(xt.reshape(128, -1))  # [128,(tck,c,k)]
        in_maps.append(dict(
            xbf=xt.astype(xdt), xres=xt.astype(bf),
            wt=wt_t, wp=wp_t, wg=wg_t, ww=ww_t,
            mask=mask, gb=gb))

    res = run_bass_kernel_spmd(nc, in_maps, list(range(N_CORES)),
                               trace=_trace, **(_trace_kwargs or {}))
    _last_results = res

    ys = []
    for b in range(B):
        o = np.asarray(res.results[b]["outy"], dtype=np.float32)
        # [128, (ct, tok)] with tok TIME-major -> [TOK, C] -> [T, N, C]
        o = o.reshape(128, NCC, TOK).transpose(2, 1, 0).reshape(TOK, C)
        ys.append(o.reshape(T, N, C))
    return np.stack(ys)


# revision 32
# speedup vs baseline: 1.0729x; 1.0184x over previous
"""Trainium2 Bass kernel for CrossInferBlock (spatial+temporal cross attention
+ out-projection + residual + BatchNorm over (B,T,N)).

Sharding: data-parallel over B across 8 NeuronCores (one batch element per
core). BN batch statistics are all-reduced across cores (8KB collective).

All matmuls run in bf16 (fp32 PSUM accumulate); residual/stats/BN in fp32.

Token orders: x and the projections (thT/phT/g) use ACTOR-MAJOR order
(tok = j*T + t) so every matmul operand is a legal single-stride access
pattern: temporal groups (8 actors x 16 timesteps) are contiguous 128-token
slices, spatial slices (one timestep, all 128 actors) are stride-T slices.
stT and the output use TIME-MAJOR order (tok = t*N + j): the temporal apply
pays a strided scatter once per group (hidden under the long projection
phase, split across ACT and DVE), which makes the phase-2 spatial adds
contiguous AND lets the out-projection start per 512-token time-chunk as
soon as its 4 spatial slices have landed.

g is projected ONCE (actor-group tiles, stationary operand read straight
from x); the 16 per-timestep tiles needed by the spatial applies are derived
with partition-gather SBUF->SBUF DMAs instead of a second projection pass.

The BN stats collective is split 7/1 so the first AllReduce's rendezvous
overlaps the out-projection tail and the bulk BN apply overlaps the second;
BN apply+store is split across the ACT and DVE engines with bf16 stores on
two HWDGE rings (the host upcasts to fp32).

All DRAM tensors are pre-tiled host-side to [128, X] exactly matching their
SBUF destination so every load is a full-row contiguous DMA.
"""

import sys

if "/opt/trn_rl_repo" not in sys.path:
    sys.path.insert(0, "/opt/trn_rl_repo")

import numpy as np
import ml_dtypes

import concourse.bass as bass
import concourse.bacc as bacc
import concourse.tile as tile
import concourse.mybir as mybir
from concourse.bass_utils import run_bass_kernel_spmd
from contextlib import ExitStack

F32 = mybir.dt.float32
BF16 = mybir.dt.bfloat16
F8 = mybir.dt.float8e4
DR = mybir.MatmulPerfMode.DoubleRow
AX = mybir.AxisListType
OP = mybir.AluOpType
ACT_FN = mybir.ActivationFunctionType

N_CORES = 8
B, T, N, C = 8, 16, 128, 1024
L = C // 2            # 512
TOK = T * N           # 2048 tokens per batch element
NTOK_GLOBAL = B * T * N
JG = 8                # actors per temporal group
NGRP = N // JG        # 16 groups
BN_EPS = 1e-5

SP_SCALE = 1.0 / (N * (T + N))   # spatial: /N then /(T+N)
TP_SCALE = 1.0 / (T * (T + N))   # temporal: /T then /(T+N)

NCC = C // 128     # 8 c-chunks
NLC = L // 128     # 4 l-chunks
NTC = TOK // 512   # 4 token chunks
CT_SPLIT = 7       # channel tiles covered by the first stats collective

_compiled = None
_last_results = None

USE_COLLECTIVE = True
USE_GATHER = True        # g_sp via SBUF->SBUF partition-gather DMA
USE_FP8 = True           # fp8e4 DoubleRow for theta/g projections
PH_BF16 = False          # phi stays bf16 (recovers quantization margin)
USE_FP8_OUT = False      # fp8e4 DoubleRow out-projection (stT + Ww in fp8)
W_SCALE = 16.0           # host premultiplies Wt/Wg (avoids fp8 subnormals)
ST_SCALE = 16.0 if USE_FP8_OUT else 1.0   # stT stored pre-scaled in fp8
# th x16, ph x16 (or x1 if PH_BF16), g x16 -> tw/sw and tp/sp carry the
# product; stT absorbs ST_SCALE
_PROJ = W_SCALE * W_SCALE * (1.0 if PH_BF16 else W_SCALE)
ATT_DESCALE = ST_SCALE / _PROJ



def ts(i, size):
    return bass.ts(i, size)


def _build():
    nc = bacc.Bacc("TRN2", target_bir_lowering=False, debug=False,
                   num_devices=N_CORES)

    # ---- DRAM I/O (pre-tiled [128, X]) ----
    # xbf rows: partition p; cols (tck, c, k): actor-major tokens,
    # x[c*128+p, tck*512+k] with tok = j*T + t
    XDT = F8 if USE_FP8 else BF16
    xbf_d = nc.dram_tensor("xbf", [128, NTC * NCC * 512], XDT,
                           kind="ExternalInput")
    xres_d = nc.dram_tensor("xres", [128, NTC * NCC * 512], BF16,
                            kind="ExternalInput")
    wt_d = nc.dram_tensor("wt", [128, NCC * L], XDT, kind="ExternalInput")
    wp_d = nc.dram_tensor("wp", [128, NCC * L],
                          BF16 if PH_BF16 else XDT, kind="ExternalInput")
    wg_d = nc.dram_tensor("wg", [128, NCC * L], XDT, kind="ExternalInput")
    ww_d = nc.dram_tensor("ww", [128, NLC * C],
                          F8 if USE_FP8_OUT else BF16, kind="ExternalInput")
    mask_d = nc.dram_tensor("mask", [128, 128], BF16, kind="ExternalInput")
    gb_d = nc.dram_tensor("gb", [128, 16], F32, kind="ExternalInput")
    # outy rows: partition p; cols (ct, tok): TIME-major tokens
    outy_d = nc.dram_tensor("outy", [128, NCC * TOK], BF16,
                            kind="ExternalOutput")

    with tile.TileContext(nc) as tc:
        with ExitStack() as outer:
            # ---------------- persistent pools ----------------
            cpool = outer.enter_context(tc.tile_pool(name="consts", bufs=1))
            wwpool = outer.enter_context(tc.tile_pool(name="wwp", bufs=1))
            stpool = outer.enter_context(tc.tile_pool(name="stp", bufs=1))
            statpool = outer.enter_context(tc.tile_pool(name="stats", bufs=1))
            pbig = outer.enter_context(
                tc.tile_pool(name="pbig", bufs=1, space="PSUM"))
            psmall = outer.enter_context(
                tc.tile_pool(name="psmall", bufs=1, space="PSUM"))
            drampool = outer.enter_context(
                tc.tile_pool(name="dramp", bufs=1, space="DRAM"))
            xbpool = outer.enter_context(tc.tile_pool(name="xbp", bufs=1))

            mask_sb = cpool.tile([128, 128], BF16, name="mask_sb",
                                 tag="mask_sb")
            gb_sb = cpool.tile([128, 16], F32, name="gb_sb", tag="gb_sb")
            ww_all = wwpool.tile([128, NLC * C],
                                 F8 if USE_FP8_OUT else BF16,
                                 name="ww_all", tag="ww")
            # stT: [p, (lc, tok)] with tok TIME-major (tok = t*N + j)
            stT = stpool.tile([128, NLC * TOK],
                              F8 if USE_FP8_OUT else BF16,
                              name="stT", tag="stT")

            stat_sum = statpool.tile([128, 32], F32, name="stat_sum",
                                     tag="stat_sum")
            stat_sq = statpool.tile([128, 32], F32, name="stat_sq",
                                    tag="stat_sq")
            red_in = statpool.tile([128, 16], F32, name="red_in", tag="red_in")
            red_out = statpool.tile([128, 16], F32, name="red_out",
                                    tag="red_out")
            scalev = statpool.tile([128, 8], F32, name="scalev", tag="scalev")
            biasv = statpool.tile([128, 8], F32, name="biasv", tag="biasv")

            cc_in = drampool.tile([128, 16], F32, name="cc_in", tag="cc_in")
            cc_out = drampool.tile([128, 16], F32, name="cc_out", tag="cc_out")
            cc_warm_in = drampool.tile([128, 1], F32, name="cc_warm_in",
                                       tag="cc_warm_in")
            cc_warm_out = drampool.tile([128, 1], F32, name="cc_warm_out",
                                        tag="cc_warm_out")
            # DRAM bounce buffer for the g repack: rows = TIME-major tokens
            g_dram = drampool.tile([TOK, L], BF16, name="g_dram",
                                   tag="g_dram")

            with ExitStack() as mid:
                thpool = mid.enter_context(tc.tile_pool(name="thp", bufs=1))
                gpool = mid.enter_context(tc.tile_pool(name="gp", bufs=1))
                attnpool = mid.enter_context(tc.tile_pool(name="attn", bufs=1))
                wg_all = gpool.tile([128, NCC * L], XDT, name="wg_all",
                                    tag="wg")

                # thT/phT: [p (l within lc), tok] ACTOR-major
                thT = [thpool.tile([128, TOK], BF16, name=f"thT{lc}",
                                   tag=f"thT{lc}") for lc in range(NLC)]
                phT = [thpool.tile([128, TOK], BF16, name=f"phT{lc}",
                                   tag=f"phT{lc}") for lc in range(NLC)]
                # g_act: one tile [128, NGRP*L]; group jg at cols jg*L..,
                # partitions = group tokens (jl, t): p = jl*T + t
                g_act = gpool.tile([128, NGRP * L], BF16, name="gact",
                                   tag="gact")
                # g_sp[i]: partitions = actors at time i (gathered via DMA)
                g_sp = [gpool.tile([128, L], BF16, name=f"gsp{i}",
                                   tag=f"gsp{i}") for i in range(T)]

                def sp_view(tile_ap, i):
                    """[128, TOK] actor-major tile -> time-i slice
                    (128 actors, stride T)."""
                    return tile_ap.rearrange("p (j t) -> p t j", t=T)[
                        :, i:i + 1, :]

                with ExitStack() as phase_a:
                    wpool = phase_a.enter_context(
                        tc.tile_pool(name="wp", bufs=1))

                    xbf = xbpool.tile([128, NTC * NCC * 512], XDT,
                                      name="xbf", tag="xbf")
                    xres = xbpool.tile([128, NCC * TOK], BF16,
                                       name="xres", tag="xres")
                    wt_all = wpool.tile([128, NCC * L], XDT, name="wt_all",
                                        tag="wt")
                    wp_all = wpool.tile([128, NCC * L],
                                        BF16 if PH_BF16 else XDT,
                                        name="wp_all", tag="wp")

                    XC = NCC * 512   # cols per token chunk

                    def xdst(tck, clo, chi):
                        # SBUF layout (tck, c, k) matches DRAM: contiguous
                        return xbf[:, tck * XC + clo * 512:
                                   tck * XC + chi * 512]

                    def xsrc(tck, clo, chi):
                        return xbf_d[:, tck * XC + clo * 512:
                                     tck * XC + chi * 512]

                    # gpsimd carries ONLY the warm-up collective: it
                    # blocks its queue until the cross-core rendezvous, so
                    # no loads may sit behind it
                    if USE_COLLECTIVE:
                        nc.gpsimd.dma_start(cc_warm_in[:], gb_d[:, 0:1])
                        nc.gpsimd.collective_compute(
                            "AllReduce", OP.add,
                            replica_groups=[list(range(N_CORES))],
                            ins=[cc_warm_in.opt()], outs=[cc_warm_out.opt()])
                    # halve the first wt/x transfers so the first matmuls
                    # start as early as possible
                    XRC = NCC * 512   # bf16 elems per xres chunk

                    def xresdst(tck):
                        return xres.rearrange("p (c k) -> p c k", c=NCC)[
                            :, :, ts(tck, 512)]

                    # critical first: wt+x0 (sync) / wp (scalar); the
                    # 4MB residual copy only feeds the out-projection, so
                    # it trails everything (the load window is HBM-
                    # contended across all 8 cores)
                    nc.sync.dma_start(wt_all[:], wt_d[:])
                    nc.scalar.dma_start(wp_all[:], wp_d[:])
                    nc.sync.dma_start(xdst(0, 0, 8), xsrc(0, 0, 8))
                    nc.scalar.dma_start(mask_sb[:], mask_d[:])
                    nc.scalar.dma_start(gb_sb[:], gb_d[:])
                    nc.scalar.dma_start(wg_all[:], wg_d[:])
                    nc.sync.dma_start(xdst(1, 0, 8), xsrc(1, 0, 8))
                    nc.sync.dma_start(xdst(2, 0, 8), xsrc(2, 0, 8))
                    nc.scalar.dma_start(xdst(3, 0, 8), xsrc(3, 0, 8))
                    nc.scalar.dma_start(ww_all[:], ww_d[:])
                    nc.sync.dma_start(xresdst(0), xres_d[:, ts(0, XRC)])
                    nc.scalar.dma_start(xresdst(1), xres_d[:, ts(1, XRC)])
                    nc.sync.dma_start(xresdst(2), xres_d[:, ts(2, XRC)])
                    nc.scalar.dma_start(xresdst(3), xres_d[:, ts(3, XRC)])

                    def xsl(tck, c, off=0, n=512):
                        base = (tck * NCC + c) * 512
                        return xbf[:, base + off:base + off + n]

                    def wsl(w_all, c, lc):
                        return w_all[:, c * L + lc * 128:c * L + (lc + 1) * 128]

                    # temporal attention; INITIALIZES stT with a strided
                    # scatter into the time-major layout (split ACT/DVE)
                    pend_tp = []   # (jg, twp)

                    def emit_tw(jg):
                        twp = psmall.tile([128, 128], F32, name="ps_tw",
                                          tag="ps_small", bufs=4)
                        for lc in range(NLC):
                            nc.tensor.matmul(twp[:], phT[lc][:, ts(jg, 128)],
                                             thT[lc][:, ts(jg, 128)],
                                             start=(lc == 0),
                                             stop=(lc == NLC - 1))
                        pend_tp.append((jg, twp))

                    def emit_tp():
                        jg, twp = pend_tp.pop(0)
                        sb = attnpool.tile([128, 128], BF16, name="sb",
                                           tag="sb", bufs=3)
                        nc.vector.tensor_mul(sb[:], twp[:], mask_sb[:])
                        pp = psmall.tile([128, 512], F32, name="ps_tp",
                                         tag="ps_small", bufs=4)
                        for lc in range(NLC):
                            nc.tensor.matmul(pp[:, ts(lc, 128)],
                                             g_act[:, jg * L + lc * 128:
                                                   jg * L + (lc + 1) * 128],
                                             sb[:])
                        # pp cols (jl, t); scatter to time-major stT:
                        # dst col for (jl, t) = t*N + jg*JG + jl
                        dst = stT.rearrange("p (a t jg jl) -> p a jl t jg",
                                            a=NLC, t=T, jl=JG)[
                            :, :, :, :, jg]
                        src = pp.rearrange("p (a jl t) -> p a jl t",
                                           a=NLC, jl=JG)
                        tp_eff = TP_SCALE * (ATT_DESCALE if USE_FP8
                                             else 1.0)
                        nc.scalar.mul(dst, src, tp_eff)

                    # ------- phase 1: projections + g_act + temporal -------
                    xpair4 = xbf.rearrange("p (a c k) -> p a c k",
                                           a=NTC, c=NCC)

                    def xres_sl(tck, c):
                        base = c * TOK + tck * 512
                        return xres[:, base:base + 512]

                    for tck in range(NTC):
                        for (w_all, dst) in ((wt_all, thT), (wp_all, phT)):
                            is_ph = w_all is wp_all
                            use8 = USE_FP8 and not (PH_BF16 and is_ph)
                            wpair = w_all.rearrange("p (c l) -> p c l",
                                                    c=NCC)
                            for lc in range(NLC):
                                ps = pbig.tile([128, 512], F32, name="ps_proj",
                                               tag="ps_big", bufs=4)
                                if use8:
                                    for ci in range(NCC // 2):
                                        nc.tensor.matmul(
                                            ps[:],
                                            wpair[:, 2 * ci:2 * ci + 2,
                                                  lc * 128:(lc + 1) * 128],
                                            xpair4[:, tck,
                                                   2 * ci:2 * ci + 2, :],
                                            start=(ci == 0),
                                            stop=(ci == NCC // 2 - 1),
                                            perf_mode=DR)
                                else:
                                    for c in range(NCC):
                                        nc.tensor.matmul(
                                            ps[:], wsl(w_all, c, lc),
                                            xres_sl(tck, c)
                                            if (PH_BF16 and is_ph)
                                            else xsl(tck, c),
                                            start=(c == 0),
                                            stop=(c == NCC - 1))
                                nc.vector.tensor_copy(
                                    dst[lc][:, ts(tck, 512)], ps[:])
                        wgpair = wg_all.rearrange("p (c l) -> p c l",
                                                  c=NCC)
                        for jg in range(4 * tck, 4 * tck + 4):
                            ps = pbig.tile([128, 512], F32, name="ps_ga",
                                           tag="ps_big", bufs=4)
                            if USE_FP8:
                                for ci in range(NCC // 2):
                                    nc.tensor.matmul(
                                        ps[:],
                                        xpair4[:, jg // 4,
                                               2 * ci:2 * ci + 2,
                                               (jg % 4) * 128:
                                               (jg % 4 + 1) * 128],
                                        wgpair[:, 2 * ci:2 * ci + 2, :],
                                        start=(ci == 0),
                                        stop=(ci == NCC // 2 - 1),
                                        perf_mode=DR)
                            else:
                                for c in range(NCC):
                                    nc.tensor.matmul(
                                        ps[:], xsl(jg // 4, c,
                                                   (jg % 4) * 128, 128),
                                        wg_all[:, ts(c, 512)],
                                        start=(c == 0), stop=(c == NCC - 1))
                            nc.vector.tensor_copy(g_act[:, ts(jg, 512)],
                                                  ps[:])
                            emit_tw(jg)
                            if len(pend_tp) >= 2:
                                emit_tp()
                    while pend_tp:
                        emit_tp()

                # ------- phase 1.5: build g_sp[i] -------
                if USE_GATHER:
                    # two-hop repack through DRAM with plain access
                    # patterns: token (t, actor jg*8+jl) sits in g_act
                    # group-block jg at partition jl*T + t.
                    # hop 1 (per jl): SBUF partitions jl*T..jl*T+T ->
                    # DRAM rows (t, jg, jl)
                    dview = g_dram.rearrange("(t jg jl) l -> t jg jl l",
                                             t=T, jg=NGRP)
                    for jl in range(JG):
                        src = g_act.rearrange("p (jg l) -> p jg l",
                                              jg=NGRP)[jl * T:(jl + 1) * T]
                        eng = nc.sync if jl % 2 == 0 else nc.scalar
                        eng.dma_start(dview[:, :, jl, :], src)
                    # hop 2 (per i): contiguous DRAM block -> g_sp[i]
                    d2 = g_dram.rearrange("(t r) l -> t r l", t=T)
                    for i in range(T):
                        eng = nc.sync if i % 2 == 0 else nc.scalar
                        eng.dma_start(g_sp[i][:], d2[i])
                else:
                    # recompute g at each timestep from x (stride-T slices)
                    for i in range(T):
                        ps = pbig.tile([128, 512], F32, name="ps_g",
                                       tag="ps_big", bufs=4)
                        for c in range(NCC):
                            xc = xbf.rearrange(
                                "p (c j t) -> p c t j", c=NCC, t=T)[
                                :, c, i:i + 1, :]
                            nc.tensor.matmul(
                                ps[:], xc, wg_all[:, ts(c, 512)],
                                start=(c == 0), stop=(c == NCC - 1))
                        nc.scalar.copy(g_sp[i][:], ps[:])

                # ------- phase 2: spatial attention + out-projection -------
                with tc.tile_pool(name="outp", bufs=1) as outpool, \
                     tc.tile_pool(name="yp", bufs=1) as ypool, \
                     tc.tile_pool(name="sqp", bufs=1) as sqpool:
                    out_sb = []
                    inv_n = 1.0 / float(NTOK_GLOBAL)
                    for ct in range(NCC):
                        out_sb.append(outpool.tile(
                            [128, TOK], BF16, name=f"out{ct}", tag=f"out{ct}"))

                    swb = []
                    # all sw matmuls first: covers the g_sp gather DMAs
                    for i in range(T):
                        swp = psmall.tile([128, 128], F32, name="ps_sw",
                                          tag="ps_small", bufs=4)
                        for lc in range(NLC):
                            nc.tensor.matmul(swp[:], sp_view(phT[lc], i),
                                             sp_view(thT[lc], i),
                                             start=(lc == 0),
                                             stop=(lc == NLC - 1))
                        b = attnpool.tile([128, 128], BF16, name=f"swb{i}",
                                          tag=f"swb{i}", bufs=1)
                        nc.vector.tensor_copy(b[:], swp[:])
                        swb.append(b)

                    def emit_sp(i):
                        pp = psmall.tile([128, 512], F32, name="ps_sp",
                                         tag="ps_small", bufs=4)
                        for lc in range(NLC):
                            nc.tensor.matmul(pp[:, ts(lc, 128)],
                                             g_sp[i][:, ts(lc, 128)],
                                             swb[i][:])
                        # contiguous read-modify-write add into stT (time-
                        # major: time-i slice is cols i*128..(i+1)*128)
                        dst = stT.rearrange("p (a k) -> p a k", a=NLC)[
                            :, :, ts(i, 128)]
                        src = pp.rearrange("p (a k) -> p a k", a=NLC)
                        nc.vector.scalar_tensor_tensor(
                            out=dst, in0=src,
                            scalar=SP_SCALE * (ATT_DESCALE if USE_FP8
                                               else 1.0),
                            in1=dst, op0=OP.mult, op1=OP.add)

                    def x_res(tck, ct):
                        """x cols for TIME-chunk tck (t in 4tck..4tck+4,
                        all j), channel chunk ct, in (t, j) order.

                        xbf col for (j, t): ct*TOK + j*T + t.
                        """
                        v = xres.rearrange("p (c j t) -> p c t j",
                                           c=NCC, t=T)
                        return v[:, ct, 4 * tck:4 * tck + 4, :]

                    def emit_outproj_chunk(tck, ct):
                        o = out_sb[ct]
                        ps = pbig.tile([128, 512], F32, name="ps_out",
                                       tag="ps_big", bufs=4)
                        if USE_FP8_OUT:
                            wwp = ww_all.rearrange("p (a c1) -> p a c1",
                                                   a=NLC)
                            stp = stT.rearrange("p (a k) -> p a k", a=NLC)
                            for j in range(NLC // 2):
                                nc.tensor.matmul(
                                    ps[:],
                                    wwp[:, 2 * j:2 * j + 2,
                                        ct * 128:(ct + 1) * 128],
                                    stp[:, 2 * j:2 * j + 2,
                                        tck * 512:(tck + 1) * 512],
                                    start=(j == 0),
                                    stop=(j == NLC // 2 - 1),
                                    perf_mode=DR)
                        else:
                            for lc in range(NLC):
                                nc.tensor.matmul(
                                    ps[:],
                                    ww_all[:, lc * C + ct * 128:
                                           lc * C + (ct + 1) * 128],
                                    stT[:, lc * TOK + tck * 512:
                                        lc * TOK + tck * 512 + 512],
                                    start=(lc == 0), stop=(lc == NLC - 1))
                        col = ct * NTC + tck
                        nc.vector.scalar_tensor_tensor(
                            out=o[:, ts(tck, 512)], in0=ps[:],
                            scalar=1.0 / (ST_SCALE * W_SCALE)
                            if USE_FP8_OUT else 1.0,
                            in1=x_res(tck, ct),
                            op0=OP.mult, op1=OP.add,
                            accum_out=stat_sum[:, col:col + 1])
                        sq = sqpool.tile([128, 512], F32, name="sqscr",
                                         tag="sq", bufs=3)
                        nc.scalar.activation(
                            sq[:], o[:, ts(tck, 512)], ACT_FN.Square,
                            accum_out=stat_sq[:, col:col + 1])

                    # pipeline: spatial applies feed out-proj chunk by chunk;
                    # last chunk is ct-ordered so the stats collective can
                    # start before ct7 finishes
                    for i in range(4):
                        emit_sp(i)
                    for tck in range(NTC - 1):
                        for ct in range(NCC):
                            emit_outproj_chunk(tck, ct)
                            if ct < 4:
                                nxt = (tck + 1) * 4 + ct
                                if nxt < T:
                                    emit_sp(nxt)

                    def emit_stats_cc():
                        """AllReduce sum+sumsq for all channel tiles."""
                        nc.vector.tensor_reduce(
                            red_in[:, 0:8],
                            stat_sum.rearrange("p (a b) -> p a b", a=8),
                            axis=AX.X, op=OP.add)
                        nc.vector.tensor_reduce(
                            red_in[:, 8:16],
                            stat_sq.rearrange("p (a b) -> p a b", a=8),
                            axis=AX.X, op=OP.add)
                        if USE_COLLECTIVE:
                            nc.gpsimd.dma_start(cc_in[:], red_in[:])
                            nc.gpsimd.collective_compute(
                                "AllReduce", OP.add,
                                replica_groups=[list(range(N_CORES))],
                                ins=[cc_in.opt()], outs=[cc_out.opt()])
                            nc.gpsimd.dma_start(red_out[:], cc_out[:])
                        else:
                            nc.vector.tensor_scalar_mul(
                                red_out[:], red_in[:], float(N_CORES))

                    def emit_bn_params(part, lo, hi):
                        n = hi - lo
                        mean = statpool.tile([128, n], F32, name=f"mean{part}",
                                             tag=f"mean{part}")
                        var = statpool.tile([128, n], F32, name=f"var{part}",
                                            tag=f"var{part}")
                        std = statpool.tile([128, n], F32, name=f"std{part}",
                                            tag=f"std{part}")
                        rstd = statpool.tile([128, n], F32, name=f"rstd{part}",
                                             tag=f"rstd{part}")
                        nc.vector.tensor_scalar_mul(mean[:],
                                                    red_out[:, lo:hi], inv_n)
                        nc.vector.tensor_scalar_mul(
                            var[:], red_out[:, 8 + lo:8 + hi], inv_n)
                        nc.vector.tensor_mul(std[:], mean[:], mean[:])
                        nc.vector.tensor_tensor(var[:], var[:], std[:],
                                                op=OP.subtract)
                        nc.vector.tensor_scalar_add(var[:], var[:], BN_EPS)
                        nc.scalar.activation(std[:], var[:], ACT_FN.Sqrt,
                                             bias=0.0)
                        nc.vector.reciprocal(rstd[:], std[:])
                        nc.vector.tensor_mul(scalev[:, lo:hi], rstd[:],
                                             gb_sb[:, lo:hi])
                        nc.vector.tensor_mul(rstd[:], mean[:],
                                             scalev[:, lo:hi])
                        nc.vector.tensor_tensor(biasv[:, lo:hi],
                                                gb_sb[:, 8 + lo:8 + hi],
                                                rstd[:], op=OP.subtract)

                    def emit_apply(ct):
                        # DVE applies (2x bf16 mode, ~0.8us/tile) + ONE ACT
                        # tile; stores round-robin on sync/scalar/gpsimd
                        src = out_sb[ct][:]
                        dst = outy_d[:, ct * TOK:(ct + 1) * TOK]
                        if ct in (3, 7):
                            y = ypool.tile([128, TOK], BF16, name="ya",
                                           tag="ya", bufs=2)
                            nc.scalar.activation(
                                y[:], src, ACT_FN.Identity,
                                scale=scalev[:, ct:ct + 1],
                                bias=biasv[:, ct:ct + 1])
                        else:
                            y = ypool.tile([128, TOK], BF16, name="yb",
                                           tag="yb", bufs=4)
                            nc.vector.tensor_scalar(
                                out=y[:], in0=src,
                                scalar1=scalev[:, ct:ct + 1],
                                scalar2=biasv[:, ct:ct + 1],
                                op0=OP.mult, op1=OP.add)
                        eng = (nc.sync, nc.scalar, nc.gpsimd)[ct % 3]
                        eng.dma_start(dst, y[:])

                    # last token chunk; single collective after all stats
                    for ct in range(NCC):
                        emit_outproj_chunk(NTC - 1, ct)
                    emit_stats_cc()
                    emit_bn_params(0, 0, NCC)
                    for ct in range(NCC):
                        emit_apply(ct)

    nc.compile()
    return nc


def _get_compiled():
    global _compiled
    if _compiled is None:
        _compiled = _build()
    return _compiled


def _tile_rows(a, nchunk):
    """[R, X] -> [128, nchunk*X] with row p, col (c*X+x) = a[c*128+p, x]."""
    R, X = a.shape
    assert R == nchunk * 128
    return np.ascontiguousarray(
        a.reshape(nchunk, 128, X).transpose(1, 0, 2).reshape(128, -1))


def kernel(x, Wt, Wp, Wg, Ww, gamma, beta, _trace=False, _trace_kwargs=None):
    global _last_results
    nc = _get_compiled()

    x = np.asarray(x, dtype=np.float32)
    Wt = np.asarray(Wt, dtype=np.float32)
    Wp = np.asarray(Wp, dtype=np.float32)
    Wg = np.asarray(Wg, dtype=np.float32)
    Ww = np.asarray(Ww, dtype=np.float32)
    gamma = np.asarray(gamma, dtype=np.float32)
    beta = np.asarray(beta, dtype=np.float32)

    bf = ml_dtypes.bfloat16
    xdt = ml_dtypes.float8_e4m3fn if USE_FP8 else bf
    wmul = W_SCALE if USE_FP8 else 1.0
    wt_t = _tile_rows(np.ascontiguousarray(Wt.T) * wmul, NCC).astype(xdt)
    wp_t = _tile_rows(np.ascontiguousarray(Wp.T)
                      * (1.0 if PH_BF16 else wmul),
                      NCC).astype(bf if PH_BF16 else xdt)
    wg_t = _tile_rows(np.ascontiguousarray(Wg.T) * wmul, NCC).astype(xdt)
    owdt = ml_dtypes.float8_e4m3fn if USE_FP8_OUT else bf
    ww_t = _tile_rows(np.ascontiguousarray(Ww.T)
                      * (W_SCALE if USE_FP8_OUT else 1.0),
                      NLC).astype(owdt)                            # [L, C]
    r = np.arange(128)
    mask = (r[:, None] // T == r[None, :] // T).astype(bf)
    gb = np.concatenate(
        [gamma.reshape(NCC, 128).T,
         beta.reshape(NCC, 128).T], axis=1).astype(np.float32)  # [128, 16]

    # actor-major token order (tok = j*T + t), cols laid out (tck, c, k)
    xa = x.transpose(0, 2, 1, 3).reshape(B, TOK, C)
    in_maps = []
    for b in range(B):
        xT = np.ascontiguousarray(xa[b].T)            # [C, TOK] f32
        xt = xT.reshape(NCC, 128, NTC, 512).transpose(1, 2, 0, 3)
        xt = np.ascontiguousarray(xt.reshape(128, -1))  # [128,(tck,c,k)]
        in_maps.append(dict(
            xbf=xt.astype(xdt), xres=xt.astype(bf),
            wt=wt_t, wp=wp_t, wg=wg_t, ww=ww_t,
            mask=mask, gb=gb))

    res = run_bass_kernel_spmd(nc, in_maps, list(range(N_CORES)),
                               trace=_trace, **(_trace_kwargs or {}))
    _last_results = res

    ys = []
    for b in range(B):
        o = np.asarray(res.results[b]["outy"], dtype=np.float32)
        # [128, (ct, tok)] with tok TIME-major -> [TOK, C] -> [T, N, C]
        o = o.reshape(128, NCC, TOK).transpose(2, 1, 0).reshape(TOK, C)
        ys.append(o.reshape(T, N, C))
    return np.stack(ys)


# revision 33
# speedup vs baseline: 1.0762x; 1.0031x over previous
"""Trainium2 Bass kernel for CrossInferBlock (spatial+temporal cross attention
+ out-projection + residual + BatchNorm over (B,T,N)).

Sharding: data-parallel over B across 8 NeuronCores (one batch element per
core). BN batch statistics are all-reduced across cores (one 8KB AllReduce).

Precision: theta/phi/g projections run in fp8e4 with DoubleRow perf mode
(two c-chunks contracted per matmul, ~1.9x the bf16 rate); weights are
pre-scaled by 16 host-side to stay out of the fp8 subnormal range and the
scale is folded into the attention epilogue constants. Attention matmuls,
the out-projection and the residual path stay bf16 (a separate bf16 copy of
x feeds the residual; fp8 x there would break the 2e-2 gate). PSUM
accumulation is fp32 everywhere; measured rel err vs the fp32 reference is
1.58e-2.

Token orders: x and the projections use ACTOR-MAJOR order (tok = j*T + t)
so every matmul operand is a legal single-free-dim access pattern: temporal
groups (8 actors x 16 timesteps) are contiguous 128-token slices, spatial
slices (one timestep, all 128 actors) are stride-T slices. stT and the
output use TIME-MAJOR order (tok = t*N + j): the temporal apply pays a
strided scatter once per group (on ACT, hidden under the long projection
phase), which makes the phase-2 spatial adds contiguous AND lets the
out-projection start per 512-token time-chunk as soon as its 4 spatial
slices have landed.

g is projected ONCE (actor-group tiles, stationary operand read straight
from x); the 16 per-timestep tiles needed by the spatial applies are
derived with a two-hop DRAM-bounce repack (8 partition-slab stores laid
out time-major + 16 contiguous reloads) instead of a second projection
pass. (Direct SBUF->SBUF partition-strided gathers hard-crash the runtime;
gpsimd compute ops do too - only its DMAs are used.)

Schedule: the warm-up collective rides an otherwise-empty gpsimd queue (it
blocks that queue until the cross-core rendezvous); loads are ordered
critical-first on the sync/scalar rings with the residual copy trailing.
Phase 1 interleaves theta/phi chunks with g + temporal attention; phase 2
pipelines spatial applies with out-projection chunks. A single stats
AllReduce follows the last out-projection tile; BN apply is split
DVE/ACT with bf16 stores round-robined on three HWDGE rings (the host
upcasts to fp32).

All DRAM tensors are pre-tiled host-side to [128, X] matching their SBUF
destination so every load is a big-packet contiguous DMA.
"""

import sys

if "/opt/trn_rl_repo" not in sys.path:
    sys.path.insert(0, "/opt/trn_rl_repo")

import numpy as np
import ml_dtypes

import concourse.bass as bass
import concourse.bacc as bacc
import concourse.tile as tile
import concourse.mybir as mybir
from concourse.bass_utils import run_bass_kernel_spmd
from contextlib import ExitStack

F32 = mybir.dt.float32
BF16 = mybir.dt.bfloat16
F8 = mybir.dt.float8e4
DR = mybir.MatmulPerfMode.DoubleRow
AX = mybir.AxisListType
OP = mybir.AluOpType
ACT_FN = mybir.ActivationFunctionType

N_CORES = 8
B, T, N, C = 8, 16, 128, 1024
L = C // 2            # 512
TOK = T * N           # 2048 tokens per batch element
NTOK_GLOBAL = B * T * N
JG = 8                # actors per temporal group
NGRP = N // JG        # 16 groups
BN_EPS = 1e-5

SP_SCALE = 1.0 / (N * (T + N))   # spatial: /N then /(T+N)
TP_SCALE = 1.0 / (T * (T + N))   # temporal: /T then /(T+N)

NCC = C // 128     # 8 c-chunks
NLC = L // 128     # 4 l-chunks
NTC = TOK // 512   # 4 token chunks
CT_SPLIT = 7       # channel tiles covered by the first stats collective

_compiled = None
_last_results = None

USE_COLLECTIVE = True
USE_GATHER = True        # g_sp via SBUF->SBUF partition-gather DMA
USE_FP8 = True           # fp8e4 DoubleRow for theta/g projections
PH_BF16 = False          # phi stays bf16 (recovers quantization margin)
USE_FP8_OUT = False      # fp8e4 DoubleRow out-projection (stT + Ww in fp8)
W_SCALE = 16.0           # host premultiplies Wt/Wg (avoids fp8 subnormals)
ST_SCALE = 16.0 if USE_FP8_OUT else 1.0   # stT stored pre-scaled in fp8
# th x16, ph x16 (or x1 if PH_BF16), g x16 -> tw/sw and tp/sp carry the
# product; stT absorbs ST_SCALE
_PROJ = W_SCALE * W_SCALE * (1.0 if PH_BF16 else W_SCALE)
ATT_DESCALE = ST_SCALE / _PROJ



def ts(i, size):
    return bass.ts(i, size)


def _build():
    nc = bacc.Bacc("TRN2", target_bir_lowering=False, debug=False,
                   num_devices=N_CORES)

    # ---- DRAM I/O (pre-tiled [128, X]) ----
    # xbf rows: partition p; cols (tck, c, k): actor-major tokens,
    # x[c*128+p, tck*512+k] with tok = j*T + t
    XDT = F8 if USE_FP8 else BF16
    xbf_d = nc.dram_tensor("xbf", [128, NTC * NCC * 512], XDT,
                           kind="ExternalInput")
    xres_d = nc.dram_tensor("xres", [128, NTC * NCC * 512], BF16,
                            kind="ExternalInput")
    wt_d = nc.dram_tensor("wt", [128, NCC * L], XDT, kind="ExternalInput")
    wp_d = nc.dram_tensor("wp", [128, NCC * L],
                          BF16 if PH_BF16 else XDT, kind="ExternalInput")
    wg_d = nc.dram_tensor("wg", [128, NCC * L], XDT, kind="ExternalInput")
    ww_d = nc.dram_tensor("ww", [128, NLC * C],
                          F8 if USE_FP8_OUT else BF16, kind="ExternalInput")
    mask_d = nc.dram_tensor("mask", [128, 128], BF16, kind="ExternalInput")
    gb_d = nc.dram_tensor("gb", [128, 16], F32, kind="ExternalInput")
    # outy rows: partition p; cols (ct, tok): TIME-major tokens
    outy_d = nc.dram_tensor("outy", [128, NCC * TOK], BF16,
                            kind="ExternalOutput")

    with tile.TileContext(nc) as tc:
        with ExitStack() as outer:
            # ---------------- persistent pools ----------------
            cpool = outer.enter_context(tc.tile_pool(name="consts", bufs=1))
            wwpool = outer.enter_context(tc.tile_pool(name="wwp", bufs=1))
            stpool = outer.enter_context(tc.tile_pool(name="stp", bufs=1))
            statpool = outer.enter_context(tc.tile_pool(name="stats", bufs=1))
            pbig = outer.enter_context(
                tc.tile_pool(name="pbig", bufs=1, space="PSUM"))
            psmall = outer.enter_context(
                tc.tile_pool(name="psmall", bufs=1, space="PSUM"))
            drampool = outer.enter_context(
                tc.tile_pool(name="dramp", bufs=1, space="DRAM"))
            xbpool = outer.enter_context(tc.tile_pool(name="xbp", bufs=1))

            mask_sb = cpool.tile([128, 128], BF16, name="mask_sb",
                                 tag="mask_sb")
            gb_sb = cpool.tile([128, 16], F32, name="gb_sb", tag="gb_sb")
            ww_all = wwpool.tile([128, NLC * C],
                                 F8 if USE_FP8_OUT else BF16,
                                 name="ww_all", tag="ww")
            # stT: [p, (lc, tok)] with tok TIME-major (tok = t*N + j)
            stT = stpool.tile([128, NLC * TOK],
                              F8 if USE_FP8_OUT else BF16,
                              name="stT", tag="stT")

            stat_sum = statpool.tile([128, 32], F32, name="stat_sum",
                                     tag="stat_sum")
            stat_sq = statpool.tile([128, 32], F32, name="stat_sq",
                                    tag="stat_sq")
            red_in = statpool.tile([128, 16], F32, name="red_in", tag="red_in")
            red_out = statpool.tile([128, 16], F32, name="red_out",
                                    tag="red_out")
            scalev = statpool.tile([128, 8], F32, name="scalev", tag="scalev")
            biasv = statpool.tile([128, 8], F32, name="biasv", tag="biasv")

            cc_in = drampool.tile([128, 16], F32, name="cc_in", tag="cc_in")
            cc_out = drampool.tile([128, 16], F32, name="cc_out", tag="cc_out")
            cc_warm_in = drampool.tile([128, 1], F32, name="cc_warm_in",
                                       tag="cc_warm_in")
            cc_warm_out = drampool.tile([128, 1], F32, name="cc_warm_out",
                                        tag="cc_warm_out")
            # DRAM bounce buffer for the g repack: rows = TIME-major tokens
            g_dram = drampool.tile([TOK, L], BF16, name="g_dram",
                                   tag="g_dram")

            with ExitStack() as mid:
                thpool = mid.enter_context(tc.tile_pool(name="thp", bufs=1))
                gpool = mid.enter_context(tc.tile_pool(name="gp", bufs=1))
                attnpool = mid.enter_context(tc.tile_pool(name="attn", bufs=1))
                wg_all = gpool.tile([128, NCC * L], XDT, name="wg_all",
                                    tag="wg")

                # thT/phT: [p (l within lc), tok] ACTOR-major
                thT = [thpool.tile([128, TOK], BF16, name=f"thT{lc}",
                                   tag=f"thT{lc}") for lc in range(NLC)]
                phT = [thpool.tile([128, TOK], BF16, name=f"phT{lc}",
                                   tag=f"phT{lc}") for lc in range(NLC)]
                # g_act: one tile [128, NGRP*L]; group jg at cols jg*L..,
                # partitions = group tokens (jl, t): p = jl*T + t
                g_act = gpool.tile([128, NGRP * L], BF16, name="gact",
                                   tag="gact")
                # g_sp[i]: partitions = actors at time i (gathered via DMA)
                g_sp = [gpool.tile([128, L], BF16, name=f"gsp{i}",
                                   tag=f"gsp{i}") for i in range(T)]

                def sp_view(tile_ap, i):
                    """[128, TOK] actor-major tile -> time-i slice
                    (128 actors, stride T)."""
                    return tile_ap.rearrange("p (j t) -> p t j", t=T)[
                        :, i:i + 1, :]

                with ExitStack() as phase_a:
                    wpool = phase_a.enter_context(
                        tc.tile_pool(name="wp", bufs=1))

                    xbf = xbpool.tile([128, NTC * NCC * 512], XDT,
                                      name="xbf", tag="xbf")
                    xres = xbpool.tile([128, NCC * TOK], BF16,
                                       name="xres", tag="xres")
                    wt_all = wpool.tile([128, NCC * L], XDT, name="wt_all",
                                        tag="wt")
                    wp_all = wpool.tile([128, NCC * L],
                                        BF16 if PH_BF16 else XDT,
                                        name="wp_all", tag="wp")

                    XC = NCC * 512   # cols per token chunk

                    def xdst(tck, clo, chi):
                        # SBUF layout (tck, c, k) matches DRAM: contiguous
                        return xbf[:, tck * XC + clo * 512:
                                   tck * XC + chi * 512]

                    def xsrc(tck, clo, chi):
                        return xbf_d[:, tck * XC + clo * 512:
                                     tck * XC + chi * 512]

                    # gpsimd carries ONLY the warm-up collective: it
                    # blocks its queue until the cross-core rendezvous, so
                    # no loads may sit behind it
                    if USE_COLLECTIVE:
                        nc.gpsimd.dma_start(cc_warm_in[:], gb_d[:, 0:1])
                        nc.gpsimd.collective_compute(
                            "AllReduce", OP.add,
                            replica_groups=[list(range(N_CORES))],
                            ins=[cc_warm_in.opt()], outs=[cc_warm_out.opt()])
                    # halve the first wt/x transfers so the first matmuls
                    # start as early as possible
                    XRC = NCC * 512   # bf16 elems per xres chunk

                    def xresdst(tck):
                        return xres.rearrange("p (c k) -> p c k", c=NCC)[
                            :, :, ts(tck, 512)]

                    # critical first: wt+x0 (sync) / wp (scalar); the
                    # 4MB residual copy only feeds the out-projection, so
                    # it trails everything (the load window is HBM-
                    # contended across all 8 cores)
                    nc.sync.dma_start(wt_all[:], wt_d[:])
                    nc.scalar.dma_start(wp_all[:], wp_d[:])
                    nc.sync.dma_start(xdst(0, 0, 8), xsrc(0, 0, 8))
                    nc.scalar.dma_start(mask_sb[:], mask_d[:])
                    nc.scalar.dma_start(gb_sb[:], gb_d[:])
                    nc.scalar.dma_start(wg_all[:], wg_d[:])
                    nc.sync.dma_start(xdst(1, 0, 8), xsrc(1, 0, 8))
                    nc.sync.dma_start(xdst(2, 0, 8), xsrc(2, 0, 8))
                    nc.scalar.dma_start(xdst(3, 0, 8), xsrc(3, 0, 8))
                    nc.scalar.dma_start(ww_all[:], ww_d[:])
                    nc.sync.dma_start(xresdst(0), xres_d[:, ts(0, XRC)])
                    nc.scalar.dma_start(xresdst(1), xres_d[:, ts(1, XRC)])
                    nc.sync.dma_start(xresdst(2), xres_d[:, ts(2, XRC)])
                    nc.scalar.dma_start(xresdst(3), xres_d[:, ts(3, XRC)])

                    def xsl(tck, c, off=0, n=512):
                        base = (tck * NCC + c) * 512
                        return xbf[:, base + off:base + off + n]

                    def wsl(w_all, c, lc):
                        return w_all[:, c * L + lc * 128:c * L + (lc + 1) * 128]

                    # temporal attention; INITIALIZES stT with a strided
                    # scatter into the time-major layout (split ACT/DVE)
                    pend_tp = []   # (jg, twp)

                    def emit_tw(jg):
                        twp = psmall.tile([128, 128], F32, name="ps_tw",
                                          tag="ps_small", bufs=4)
                        for lc in range(NLC):
                            nc.tensor.matmul(twp[:], phT[lc][:, ts(jg, 128)],
                                             thT[lc][:, ts(jg, 128)],
                                             start=(lc == 0),
                                             stop=(lc == NLC - 1))
                        pend_tp.append((jg, twp))

                    def emit_tp():
                        jg, twp = pend_tp.pop(0)
                        sb = attnpool.tile([128, 128], BF16, name="sb",
                                           tag="sb", bufs=3)
                        nc.vector.tensor_mul(sb[:], twp[:], mask_sb[:])
                        pp = psmall.tile([128, 512], F32, name="ps_tp",
                                         tag="ps_small", bufs=4)
                        for lc in range(NLC):
                            nc.tensor.matmul(pp[:, ts(lc, 128)],
                                             g_act[:, jg * L + lc * 128:
                                                   jg * L + (lc + 1) * 128],
                                             sb[:])
                        # pp cols (jl, t); scatter to time-major stT:
                        # dst col for (jl, t) = t*N + jg*JG + jl
                        dst = stT.rearrange("p (a t jg jl) -> p a jl t jg",
                                            a=NLC, t=T, jl=JG)[
                            :, :, :, :, jg]
                        src = pp.rearrange("p (a jl t) -> p a jl t",
                                           a=NLC, jl=JG)
                        tp_eff = TP_SCALE * (ATT_DESCALE if USE_FP8
                                             else 1.0)
                        nc.scalar.mul(dst, src, tp_eff)

                    # ------- phase 1: projections + g_act + temporal -------
                    xpair4 = xbf.rearrange("p (a c k) -> p a c k",
                                           a=NTC, c=NCC)

                    def xres_sl(tck, c):
                        base = c * TOK + tck * 512
                        return xres[:, base:base + 512]

                    for tck in range(NTC):
                        for (w_all, dst) in ((wt_all, thT), (wp_all, phT)):
                            is_ph = w_all is wp_all
                            use8 = USE_FP8 and not (PH_BF16 and is_ph)
                            wpair = w_all.rearrange("p (c l) -> p c l",
                                                    c=NCC)
                            for lc in range(NLC):
                                ps = pbig.tile([128, 512], F32, name="ps_proj",
                                               tag="ps_big", bufs=4)
                                if use8:
                                    for ci in range(NCC // 2):
                                        nc.tensor.matmul(
                                            ps[:],
                                            wpair[:, 2 * ci:2 * ci + 2,
                                                  lc * 128:(lc + 1) * 128],
                                            xpair4[:, tck,
                                                   2 * ci:2 * ci + 2, :],
                                            start=(ci == 0),
                                            stop=(ci == NCC // 2 - 1),
                                            perf_mode=DR)
                                else:
                                    for c in range(NCC):
                                        nc.tensor.matmul(
                                            ps[:], wsl(w_all, c, lc),
                                            xres_sl(tck, c)
                                            if (PH_BF16 and is_ph)
                                            else xsl(tck, c),
                                            start=(c == 0),
                                            stop=(c == NCC - 1))
                                nc.vector.tensor_copy(
                                    dst[lc][:, ts(tck, 512)], ps[:])
                        wgpair = wg_all.rearrange("p (c l) -> p c l",
                                                  c=NCC)
                        for jg in range(4 * tck, 4 * tck + 4):
                            ps = pbig.tile([128, 512], F32, name="ps_ga",
                                           tag="ps_big", bufs=4)
                            if USE_FP8:
                                for ci in range(NCC // 2):
                                    nc.tensor.matmul(
                                        ps[:],
                                        xpair4[:, jg // 4,
                                               2 * ci:2 * ci + 2,
                                               (jg % 4) * 128:
                                               (jg % 4 + 1) * 128],
                                        wgpair[:, 2 * ci:2 * ci + 2, :],
                                        start=(ci == 0),
                                        stop=(ci == NCC // 2 - 1),
                                        perf_mode=DR)
                            else:
                                for c in range(NCC):
                                    nc.tensor.matmul(
                                        ps[:], xsl(jg // 4, c,
                                                   (jg % 4) * 128, 128),
                                        wg_all[:, ts(c, 512)],
                                        start=(c == 0), stop=(c == NCC - 1))
                            nc.vector.tensor_copy(g_act[:, ts(jg, 512)],
                                                  ps[:])
                            emit_tw(jg)
                            if len(pend_tp) >= 2:
                                emit_tp()
                    while pend_tp:
                        emit_tp()

                # ------- phase 1.5: build g_sp[i] -------
                if USE_GATHER:
                    # two-hop repack through DRAM with plain access
                    # patterns: token (t, actor jg*8+jl) sits in g_act
                    # group-block jg at partition jl*T + t.
                    # hop 1 (per jl): SBUF partitions jl*T..jl*T+T ->
                    # DRAM rows (t, jg, jl)
                    dview = g_dram.rearrange("(t jg jl) l -> t jg jl l",
                                             t=T, jg=NGRP)
                    for jl in range(JG):
                        src = g_act.rearrange("p (jg l) -> p jg l",
                                              jg=NGRP)[jl * T:(jl + 1) * T]
                        eng = nc.sync if jl % 2 == 0 else nc.scalar
                        eng.dma_start(dview[:, :, jl, :], src)
                    # hop 2 (per i): contiguous DRAM block -> g_sp[i]
                    d2 = g_dram.rearrange("(t r) l -> t r l", t=T)
                    for i in range(T):
                        eng = nc.sync if i % 2 == 0 else nc.scalar
                        eng.dma_start(g_sp[i][:], d2[i])
                else:
                    # recompute g at each timestep from x (stride-T slices)
                    for i in range(T):
                        ps = pbig.tile([128, 512], F32, name="ps_g",
                                       tag="ps_big", bufs=4)
                        for c in range(NCC):
                            xc = xbf.rearrange(
                                "p (c j t) -> p c t j", c=NCC, t=T)[
                                :, c, i:i + 1, :]
                            nc.tensor.matmul(
                                ps[:], xc, wg_all[:, ts(c, 512)],
                                start=(c == 0), stop=(c == NCC - 1))
                        nc.scalar.copy(g_sp[i][:], ps[:])

                # ------- phase 2: spatial attention + out-projection -------
                with tc.tile_pool(name="outp", bufs=1) as outpool, \
                     tc.tile_pool(name="yp", bufs=1) as ypool, \
                     tc.tile_pool(name="sqp", bufs=1) as sqpool:
                    out_sb = []
                    inv_n = 1.0 / float(NTOK_GLOBAL)
                    for ct in range(NCC):
                        out_sb.append(outpool.tile(
                            [128, TOK], BF16, name=f"out{ct}", tag=f"out{ct}"))

                    swb = []
                    # all sw matmuls first: covers the g_sp gather DMAs
                    for i in range(T):
                        swp = psmall.tile([128, 128], F32, name="ps_sw",
                                          tag="ps_small", bufs=4)
                        for lc in range(NLC):
                            nc.tensor.matmul(swp[:], sp_view(phT[lc], i),
                                             sp_view(thT[lc], i),
                                             start=(lc == 0),
                                             stop=(lc == NLC - 1))
                        b = attnpool.tile([128, 128], BF16, name=f"swb{i}",
                                          tag=f"swb{i}", bufs=1)
                        nc.vector.tensor_copy(b[:], swp[:])
                        swb.append(b)

                    def emit_sp(i):
                        pp = psmall.tile([128, 512], F32, name="ps_sp",
                                         tag="ps_small", bufs=4)
                        for lc in range(NLC):
                            nc.tensor.matmul(pp[:, ts(lc, 128)],
                                             g_sp[i][:, ts(lc, 128)],
                                             swb[i][:])
                        # contiguous read-modify-write add into stT (time-
                        # major: time-i slice is cols i*128..(i+1)*128)
                        dst = stT.rearrange("p (a k) -> p a k", a=NLC)[
                            :, :, ts(i, 128)]
                        src = pp.rearrange("p (a k) -> p a k", a=NLC)
                        nc.vector.scalar_tensor_tensor(
                            out=dst, in0=src,
                            scalar=SP_SCALE * (ATT_DESCALE if USE_FP8
                                               else 1.0),
                            in1=dst, op0=OP.mult, op1=OP.add)

                    def x_res(tck, ct):
                        """x cols for TIME-chunk tck (t in 4tck..4tck+4,
                        all j), channel chunk ct, in (t, j) order.

                        xbf col for (j, t): ct*TOK + j*T + t.
                        """
                        v = xres.rearrange("p (c j t) -> p c t j",
                                           c=NCC, t=T)
                        return v[:, ct, 4 * tck:4 * tck + 4, :]

                    def emit_outproj_chunk(tck, ct):
                        o = out_sb[ct]
                        ps = pbig.tile([128, 512], F32, name="ps_out",
                                       tag="ps_big", bufs=4)
                        if USE_FP8_OUT:
                            wwp = ww_all.rearrange("p (a c1) -> p a c1",
                                                   a=NLC)
                            stp = stT.rearrange("p (a k) -> p a k", a=NLC)
                            for j in range(NLC // 2):
                                nc.tensor.matmul(
                                    ps[:],
                                    wwp[:, 2 * j:2 * j + 2,
                                        ct * 128:(ct + 1) * 128],
                                    stp[:, 2 * j:2 * j + 2,
                                        tck * 512:(tck + 1) * 512],
                                    start=(j == 0),
                                    stop=(j == NLC // 2 - 1),
                                    perf_mode=DR)
                        else:
                            for lc in range(NLC):
                                nc.tensor.matmul(
                                    ps[:],
                                    ww_all[:, lc * C + ct * 128:
                                           lc * C + (ct + 1) * 128],
                                    stT[:, lc * TOK + tck * 512:
                                        lc * TOK + tck * 512 + 512],
                                    start=(lc == 0), stop=(lc == NLC - 1))
                        col = ct * NTC + tck
                        nc.vector.scalar_tensor_tensor(
                            out=o[:, ts(tck, 512)], in0=ps[:],
                            scalar=1.0 / (ST_SCALE * W_SCALE)
                            if USE_FP8_OUT else 1.0,
                            in1=x_res(tck, ct),
                            op0=OP.mult, op1=OP.add,
                            accum_out=stat_sum[:, col:col + 1])
                        sq = sqpool.tile([128, 512], F32, name="sqscr",
                                         tag="sq", bufs=3)
                        nc.scalar.activation(
                            sq[:], o[:, ts(tck, 512)], ACT_FN.Square,
                            accum_out=stat_sq[:, col:col + 1])

                    # pipeline: spatial applies feed out-proj chunk by chunk;
                    # last chunk is ct-ordered so the stats collective can
                    # start before ct7 finishes
                    for i in range(4):
                        emit_sp(i)
                    for tck in range(NTC - 1):
                        for ct in range(NCC):
                            emit_outproj_chunk(tck, ct)
                            if ct < 4:
                                nxt = (tck + 1) * 4 + ct
                                if nxt < T:
                                    emit_sp(nxt)

                    def emit_stats_cc():
                        """AllReduce sum+sumsq for all channel tiles."""
                        nc.vector.tensor_reduce(
                            red_in[:, 0:8],
                            stat_sum.rearrange("p (a b) -> p a b", a=8),
                            axis=AX.X, op=OP.add)
                        nc.vector.tensor_reduce(
                            red_in[:, 8:16],
                            stat_sq.rearrange("p (a b) -> p a b", a=8),
                            axis=AX.X, op=OP.add)
                        if USE_COLLECTIVE:
                            nc.gpsimd.dma_start(cc_in[:], red_in[:])
                            nc.gpsimd.collective_compute(
                                "AllReduce", OP.add,
                                replica_groups=[list(range(N_CORES))],
                                ins=[cc_in.opt()], outs=[cc_out.opt()])
                            nc.gpsimd.dma_start(red_out[:], cc_out[:])
                        else:
                            nc.vector.tensor_scalar_mul(
                                red_out[:], red_in[:], float(N_CORES))

                    def emit_bn_params(part, lo, hi):
                        n = hi - lo
                        mean = statpool.tile([128, n], F32, name=f"mean{part}",
                                             tag=f"mean{part}")
                        var = statpool.tile([128, n], F32, name=f"var{part}",
                                            tag=f"var{part}")
                        std = statpool.tile([128, n], F32, name=f"std{part}",
                                            tag=f"std{part}")
                        rstd = statpool.tile([128, n], F32, name=f"rstd{part}",
                                             tag=f"rstd{part}")
                        nc.vector.tensor_scalar_mul(mean[:],
                                                    red_out[:, lo:hi], inv_n)
                        nc.vector.tensor_scalar_mul(
                            var[:], red_out[:, 8 + lo:8 + hi], inv_n)
                        nc.vector.tensor_mul(std[:], mean[:], mean[:])
                        nc.vector.tensor_tensor(var[:], var[:], std[:],
                                                op=OP.subtract)
                        nc.vector.tensor_scalar_add(var[:], var[:], BN_EPS)
                        nc.scalar.activation(std[:], var[:], ACT_FN.Sqrt,
                                             bias=0.0)
                        nc.vector.reciprocal(rstd[:], std[:])
                        nc.vector.tensor_mul(scalev[:, lo:hi], rstd[:],
                                             gb_sb[:, lo:hi])
                        nc.vector.tensor_mul(rstd[:], mean[:],
                                             scalev[:, lo:hi])
                        nc.vector.tensor_tensor(biasv[:, lo:hi],
                                                gb_sb[:, 8 + lo:8 + hi],
                                                rstd[:], op=OP.subtract)

                    def emit_apply(ct):
                        # DVE applies (2x bf16 mode, ~0.8us/tile) + ONE ACT
                        # tile; stores round-robin on sync/scalar/gpsimd
                        src = out_sb[ct][:]
                        dst = outy_d[:, ct * TOK:(ct + 1) * TOK]
                        if ct in (3, 7):
                            y = ypool.tile([128, TOK], BF16, name="ya",
                                           tag="ya", bufs=2)
                            nc.scalar.activation(
                                y[:], src, ACT_FN.Identity,
                                scale=scalev[:, ct:ct + 1],
                                bias=biasv[:, ct:ct + 1])
                        else:
                            y = ypool.tile([128, TOK], BF16, name="yb",
                                           tag="yb", bufs=4)
                            nc.vector.tensor_scalar(
                                out=y[:], in0=src,
                                scalar1=scalev[:, ct:ct + 1],
                                scalar2=biasv[:, ct:ct + 1],
                                op0=OP.mult, op1=OP.add)
                        eng = (nc.sync, nc.scalar, nc.gpsimd)[ct % 3]
                        eng.dma_start(dst, y[:])

                    # last token chunk; single collective after all stats
                    for ct in range(NCC):
                        emit_outproj_chunk(NTC - 1, ct)
                    emit_stats_cc()
                    emit_bn_params(0, 0, NCC)
                    for ct in range(NCC):
                        emit_apply(ct)

    nc.compile()
    return nc


def _get_compiled():
    global _compiled
    if _compiled is None:
        _compiled = _build()
    return _compiled


def _tile_rows(a, nchunk):
    """[R, X] -> [128, nchunk*X] with row p, col (c*X+x) = a[c*128+p, x]."""
    R, X = a.shape
    assert R == nchunk * 128
    return np.ascontiguousarray(
        a.reshape(nchunk, 128, X).transpose(1, 0, 2).reshape(128, -1))


def kernel(x, Wt, Wp, Wg, Ww, gamma, beta, _trace=False, _trace_kwargs=None):
    global _last_results
    nc = _get_compiled()

    x = np.asarray(x, dtype=np.float32)
    Wt = np.asarray(Wt, dtype=np.float32)
    Wp = np.asarray(Wp, dtype=np.float32)
    Wg = np.asarray(Wg, dtype=np.float32)
    Ww = np.asarray(Ww, dtype=np.float32)
    gamma = np.asarray(gamma, dtype=np.float32)
    beta = np.asarray(beta, dtype=np.float32)

    bf = ml_dtypes.bfloat16
    xdt = ml_dtypes.float8_e4m3fn if USE_FP8 else bf
    wmul = W_SCALE if USE_FP8 else 1.0
    wt_t = _tile_rows(np.ascontiguousarray(Wt.T) * wmul, NCC).astype(xdt)
    wp_t = _tile_rows(np.ascontiguousarray(Wp.T)
                      * (1.0 if PH_BF16 else wmul),
                      NCC).astype(bf if PH_BF16 else xdt)
    wg_t = _tile_rows(np.ascontiguousarray(Wg.T) * wmul, NCC).astype(xdt)
    owdt = ml_dtypes.float8_e4m3fn if USE_FP8_OUT else bf
    ww_t = _tile_rows(np.ascontiguousarray(Ww.T)
                      * (W_SCALE if USE_FP8_OUT else 1.0),
                      NLC).astype(owdt)                            # [L, C]
    r = np.arange(128)
    mask = (r[:, None] // T == r[None, :] // T).astype(bf)
    gb = np.concatenate(
        [gamma.reshape(NCC, 128).T,
         beta.reshape(NCC, 128).T], axis=1).astype(np.float32)  # [128, 16]

    # actor-major token order (tok = j*T + t), cols laid out (tck, c, k)
    xa = x.transpose(0, 2, 1, 3).reshape(B, TOK, C)
    in_maps = []
    for b in range(B):
        xT = np.ascontiguousarray(xa[b].T)            # [C, TOK] f32
        xt = xT.reshape(NCC, 128, NTC, 512).transpose(1, 2, 0, 3)
        xt = np.ascontiguousarray(xt.reshape(128, -1))  # [128,(tck,c,k)]
        in_maps.append(dict(
            xbf=xt.astype(xdt), xres=xt.astype(bf),
            wt=wt_t, wp=wp_t, wg=wg_t, ww=ww_t,
            mask=mask, gb=gb))

    res = run_bass_kernel_spmd(nc, in_maps, list(range(N_CORES)),
                               trace=_trace, **(_trace_kwargs or {}))
    _last_results = res

    ys = []
    for b in range(B):
        o = np.asarray(res.results[b]["outy"], dtype=np.float32)
        # [128, (ct, tok)] with tok TIME-major -> [TOK, C] -> [T, N, C]
        o = o.reshape(128, NCC, TOK).transpose(2, 1, 0).reshape(TOK, C)
        ys.append(o.reshape(T, N, C))
    return np.stack(ys)
